# revision 7
# baseline (speedup 1.0000x reference)
"""Trainium2 Bass kernel for nn_CSDKM_66417374265458 (dense_cnn).

Data-parallel over batch B=8 across 8 NeuronCores (one image per core, all
parameters replicated). The only cross-core communication is a 2KB AllReduce
of the BatchNorm batch statistics (sum / sum-of-squares per channel).

Per-core pipeline (all shapes per batch element):
  c4 (256,64,64), c5 (512,32,32)
  c4_proc = conv3x3(c4)                  -> shifted-window matmuls on PE
  c5_proc = conv1x1(c5) at 32x32, then nearest-upsample (1x1 conv commutes
            with nearest upsampling)
  fused   = c4_proc + up(c5_proc)        -> fused into the PSUM->SBUF move
  y       = conv1x1(fused); BN batch stats -> AllReduce -> X = silu(s*y+b)
  sim/gate path: adaptive pools as rectangle reductions, w_sim4^T w_sim5
            folded on host, tiny matmuls + softmax -> per-region 3x3 kernels
  dynfilter: out = sum_k kern[region,k] * shift_k(X) -> scaled-identity
            matmuls on PE accumulating in PSUM (region-rect free APs)
  out     = dynfilter(X) + conv1x1(conv1x1(fused))
"""
import sys

sys.path.insert(0, "/opt/trn_rl_repo")

import numpy as np

import concourse.bass as bass  # noqa: F401  (engine types referenced via nc)
import concourse.bacc as bacc
import concourse.tile as tile
from concourse import mybir
from concourse.bass_utils import run_bass_kernel_spmd

F32 = mybir.dt.float32
F32R = mybir.dt.float32r
ALU = mybir.AluOpType
ACTF = mybir.ActivationFunctionType
AX = mybir.AxisListType

B, C4, C5, H, W = 8, 256, 512, 64, 64
OC, FR, HID = 256, 128, 16
S, K2 = 3, 9
EPS = 1e-5
NCORES = 8
NPIX = H * W  # 4096
NSTAT = float(B * NPIX)  # BN sample count per channel

# Output-space region bands (start, len) for rows and cols: pidx regions.
BANDS = [(0, 22), (22, 21), (43, 21)]
# pool4 bins on the 64x64 grid (overlapping 22-wide intervals).
P4B = [(0, 22), (21, 22), (42, 22)]
# pool5 on the 32x32 grid: the upsampled 22-wide bin maps to interval sums
# over c5 rows; bin i = sum over listed (start, count) intervals, and a
# host-folded factor (uniform bins count each row twice).
P5IV = {0: [(0, 11)], 1: [(10, 12), (11, 10)], 2: [(21, 11)]}
P5FAC = {0: 2.0, 1: 1.0, 2: 2.0}

_CACHE = {}


def _build():
    nc = bacc.Bacc("TRN2", target_bir_lowering=False, debug=False,
                   num_devices=NCORES)

    # ---- DRAM I/O -------------------------------------------------------
    c4d = nc.dram_tensor("c4", [C4, NPIX], F32, kind="ExternalInput").ap()
    c5d = nc.dram_tensor("c5", [C5, 1024], F32, kind="ExternalInput").ap()
    wc4d = nc.dram_tensor("wc4t", [C4, 9, OC], F32, kind="ExternalInput").ap()
    wc1d = nc.dram_tensor("wc1t", [C5, OC], F32, kind="ExternalInput").ap()
    wtfd = nc.dram_tensor("wtft", [C4, OC], F32, kind="ExternalInput").ap()
    wrsd = nc.dram_tensor("wrst", [C4, FR], F32, kind="ExternalInput").ap()
    wprd = nc.dram_tensor("wprt", [FR, OC], F32, kind="ExternalInput").ap()
    mtd = nc.dram_tensor("mt", [C5, C4], F32, kind="ExternalInput").ap()
    w1d = nc.dram_tensor("w1", [HID], F32, kind="ExternalInput").ap()
    b1d = nc.dram_tensor("b1", [HID], F32, kind="ExternalInput").ap()
    w2d = nc.dram_tensor("w2t", [HID, K2], F32, kind="ExternalInput").ap()
    b2d = nc.dram_tensor("b2t", [K2, K2], F32, kind="ExternalInput").ap()
    sgd = nc.dram_tensor("sgp", [K2], F32, kind="ExternalInput").ap()
    gmd = nc.dram_tensor("gam", [OC], F32, kind="ExternalInput").ap()
    btd = nc.dram_tensor("bet", [OC], F32, kind="ExternalInput").ap()
    eyd = nc.dram_tensor("i128", [128, 128], F32, kind="ExternalInput").ap()
    zpd = nc.dram_tensor("zpad", [66], F32, kind="ExternalInput").ap()
    outd = nc.dram_tensor("o_out", [OC, NPIX], F32, kind="ExternalOutput").ap()

    with tile.TileContext(nc) as tc:
        with (
            tc.tile_pool(name="big", bufs=3) as big,
            tc.tile_pool(name="ypool", bufs=2) as ypool,
            tc.tile_pool(name="pad", bufs=2) as pad,
            tc.tile_pool(name="c5pool", bufs=6) as c5pool,
            tc.tile_pool(name="wts", bufs=1) as wts,
            tc.tile_pool(name="small", bufs=1) as small,
            tc.tile_pool(name="scr", bufs=2) as scr,
            tc.tile_pool(name="idp", bufs=6) as idp,
            tc.tile_pool(name="pmain", bufs=3, space="PSUM") as pmain,
            tc.tile_pool(name="pdyn", bufs=3, space="PSUM") as pdyn,
            tc.tile_pool(name="ptiny", bufs=2, space="PSUM") as ptiny,
            tc.tile_pool(name="dram", bufs=1, space="DRAM") as dram,
        ):
            dma = nc.sync.dma_start

            # ---- weights / consts in --------------------------------
            wc4_sb = []
            for icb in range(2):
                t = wts.tile([128, 9, OC], F32, tag=f"wc4_{icb}")
                dma(t[:].bitcast(F32R), wc4d[icb * 128:(icb + 1) * 128].bitcast(F32R))
                wc4_sb.append(t)
            wc1_sb = wts.tile([128, 4, OC], F32, tag="wc1")
            dma(wc1_sb[:].bitcast(F32R), wc1d.rearrange("(b p) o -> p b o", p=128).bitcast(F32R))
            wtf_sb = wts.tile([128, 2, OC], F32, tag="wtf")
            dma(wtf_sb[:].bitcast(F32R), wtfd.rearrange("(b p) o -> p b o", p=128).bitcast(F32R))
            wrs_sb = wts.tile([128, 2, FR], F32, tag="wrs")
            dma(wrs_sb[:].bitcast(F32R), wrsd.rearrange("(b p) o -> p b o", p=128).bitcast(F32R))
            wpr_sb = wts.tile([128, OC], F32, tag="wpr")
            dma(wpr_sb[:].bitcast(F32R), wprd.bitcast(F32R))
            mt_sb = wts.tile([128, 4, C4], F32, tag="mt")
            dma(mt_sb[:], mtd.rearrange("(b p) o -> p b o", p=128))
            eye_sb = wts.tile([128, 128], F32, tag="eye")
            dma(eye_sb[:], eyd)
            w1_sb = wts.tile([1, HID], F32, tag="w1")
            dma(w1_sb[:], w1d[None, :])
            b1_sb = wts.tile([HID, 1], F32, tag="b1")
            dma(b1_sb[:], b1d[:, None])
            w2_sb = wts.tile([HID, K2], F32, tag="w2")
            dma(w2_sb[:], w2d)
            b2_sb = wts.tile([K2, K2], F32, tag="b2")
            dma(b2_sb[:], b2d)
            sg_sb = wts.tile([1, K2], F32, tag="sg")
            dma(sg_sb[:], sgd[None, :])
            gam_sb, bet_sb = [], []
            for cb in range(2):
                g = wts.tile([128, 1], F32, tag=f"gam{cb}")
                dma(g[:], gmd[cb * 128:(cb + 1) * 128][:, None])
                gam_sb.append(g)
                bt = wts.tile([128, 1], F32, tag=f"bet{cb}")
                dma(bt[:], btd[cb * 128:(cb + 1) * 128][:, None])
                bet_sb.append(bt)
            ones_sb = wts.tile([128, 1], F32, tag="ones")
            nc.vector.memset(ones_sb[:], 1.0)

            # ---- data in: c4 padded (66x66, zero ring), c5 plain ----
            c4p = []
            for cb in range(2):
                t = pad.tile([128, 66, 66], F32, tag="pad66")
                dma(t[:, 0, :].bitcast(F32R),
                    zpd[None, :].broadcast_to([128, 66]).bitcast(F32R))
                dma(t[:, 65, :].bitcast(F32R),
                    zpd[None, :].broadcast_to([128, 66]).bitcast(F32R))
                dma(t[:, 1:65, 0:1].bitcast(F32R),
                    zpd[None, :64, None].broadcast_to([128, 64, 1]).bitcast(F32R))
                dma(t[:, 1:65, 65:66].bitcast(F32R),
                    zpd[None, :64, None].broadcast_to([128, 64, 1]).bitcast(F32R))
                dma(t[:, 1:65, 1:65].bitcast(F32R),
                    c4d[cb * 128:(cb + 1) * 128].rearrange("p (h w) -> p h w", h=H).bitcast(F32R))
                c4p.append(t)
            c5_sb = []
            for icb in range(4):
                t = c5pool.tile([128, 1024], F32, tag="c5in", bufs=4)
                dma(t[:].bitcast(F32R), c5d[icb * 128:(icb + 1) * 128].bitcast(F32R))
                c5_sb.append(t)

            # ---- pool4: 9 overlapping 22x22 rect sums per ch block ---
            praw4 = []
            for cb in range(2):
                p4 = small.tile([128, K2], F32, tag=f"praw4_{cb}")
                for i, (r0, nr) in enumerate(P4B):
                    for j, (c0, ncc) in enumerate(P4B):
                        nc.vector.tensor_reduce(
                            p4[:, i * 3 + j: i * 3 + j + 1],
                            c4p[cb][:, r0 + 1:r0 + 1 + nr, c0 + 1:c0 + 1 + ncc],
                            AX.XY, ALU.add)
                praw4.append(p4)

            # ---- pool5: separable interval sums on the 32x32 grid ----
            praw5 = []
            for icb in range(4):
                v = c5_sb[icb][:].rearrange("p (h w) -> p h w", h=32)
                cs = small.tile([128, 3, 32], F32, tag=f"cs_{icb}")
                for j in range(3):
                    ivs = P5IV[j]
                    nc.vector.tensor_reduce(
                        cs[:, j, :][:, :, None], v[:, :, ivs[0][0]:ivs[0][0] + ivs[0][1]],
                        AX.X, ALU.add)
                    if len(ivs) > 1:
                        tmp = small.tile([128, 32], F32, tag=f"cstmp_{icb}")
                        nc.vector.tensor_reduce(
                            tmp[:, :, None], v[:, :, ivs[1][0]:ivs[1][0] + ivs[1][1]],
                            AX.X, ALU.add)
                        nc.vector.tensor_add(cs[:, j, :], cs[:, j, :], tmp[:])
                p5 = small.tile([128, K2], F32, tag=f"praw5_{icb}")
                for i in range(3):
                    ivs = P5IV[i]
                    for j in range(3):
                        sl = p5[:, i * 3 + j: i * 3 + j + 1]
                        nc.vector.tensor_reduce(
                            sl, cs[:, j, ivs[0][0]:ivs[0][0] + ivs[0][1]],
                            AX.X, ALU.add)
                        if len(ivs) > 1:
                            t1 = small.tile([128, 1], F32, tag=f"p5tmp_{icb}")
                            nc.vector.tensor_reduce(
                                t1[:], cs[:, j, ivs[1][0]:ivs[1][0] + ivs[1][1]],
                                AX.X, ALU.add)
                            nc.vector.tensor_add(sl, sl, t1[:])
                praw5.append(p5)

            # ---- c5 1x1 conv at 32x32 -------------------------------
            c5p_sb = []
            for cb in range(2):
                t = c5pool.tile([128, 1024], F32, tag="c5p", bufs=2)
                for pt in range(2):
                    ps = pmain.tile([128, 512], F32, tag="ps")
                    for icb in range(4):
                        nc.tensor.matmul(
                            ps[:],
                            wc1_sb[:, icb, cb * 128:(cb + 1) * 128].bitcast(F32R),
                            c5_sb[icb][:, pt * 512:(pt + 1) * 512].bitcast(F32R),
                            start=(icb == 0), stop=(icb == 3))
                    nc.scalar.copy(t[:, pt * 512:(pt + 1) * 512], ps[:])
                c5p_sb.append(t)

            # ---- materialize nearest-upsampled c5_proc ---------------
            # c5up shares slots with the later y tiles (disjoint lifetimes).
            c5up = []
            for cb in range(2):
                u = ypool.tile([128, NPIX], F32, tag="y", name=f"c5up{cb}")
                uv = u[:].rearrange("p (h a w b) -> p h a w b", h=32, a=2, b=2)
                sv = c5p_sb[cb][:].rearrange("p (h w) -> p h w", h=32)
                for a in range(2):
                    for bb in range(2):
                        nc.vector.tensor_copy(uv[:, :, a, :, bb], sv)
                c5up.append(u)

            # ---- conv3x3 + fused assembly (+ upsampled c5_proc) ------
            fused = []
            for cb in range(2):
                f = big.tile([128, NPIX], F32, tag="mid16")
                for pt in range(8):
                    ps = pmain.tile([128, 512], F32, tag="ps")
                    for icb in range(2):
                        for tap in range(9):
                            dy, dx = tap // 3, tap % 3
                            nc.tensor.matmul(
                                ps[:],
                                wc4_sb[icb][:, tap, cb * 128:(cb + 1) * 128].bitcast(F32R),
                                c4p[icb][:, pt * 8 + dy:pt * 8 + dy + 8, dx:dx + 64].bitcast(F32R),
                                start=(icb == 0 and tap == 0),
                                stop=(icb == 1 and tap == 8))
                    nc.vector.scalar_tensor_tensor(
                        out=f[:, pt * 512:(pt + 1) * 512].bitcast(F32R),
                        in0=ps[:],
                        scalar=1.0, in1=c5up[cb][:, pt * 512:(pt + 1) * 512],
                        op0=ALU.mult, op1=ALU.add)
                fused.append(f)

            # ---- to_fuse 1x1 conv -> y (+ stats partials) ------------
            y_sb, ysum_p, ysq_p = [], [], []
            for cb in range(2):
                y = ypool.tile([128, NPIX], F32, tag="y")
                su = small.tile([128, 8], F32, tag=f"ysum_{cb}")
                sq = small.tile([128, 8], F32, tag=f"ysq_{cb}")
                for pt in range(8):
                    ps = pmain.tile([128, 512], F32, tag="ps")
                    for icb in range(2):
                        nc.tensor.matmul(
                            ps[:],
                            wtf_sb[:, icb, cb * 128:(cb + 1) * 128].bitcast(F32R),
                            fused[icb][:, pt * 512:(pt + 1) * 512].bitcast(F32R),
                            start=(icb == 0), stop=(icb == 1))
                    nc.scalar.activation(
                        y[:, pt * 512:(pt + 1) * 512], ps[:], ACTF.Copy,
                        accum_out=su[:, pt:pt + 1])
                    sc = scr.tile([128, 512], F32, tag="sqscr")
                    nc.scalar.activation(sc[:], ps[:], ACTF.Square,
                                         accum_out=sq[:, pt:pt + 1])
                y_sb.append(y)
                ysum_p.append(su)
                ysq_p.append(sq)

            # ---- sim / gating / per-region kernels -------------------
            mp5_ps = []
            for cb in range(2):
                mp = ptiny.tile([128, K2], F32, tag="pt")
                for icb in range(4):
                    nc.tensor.matmul(
                        mp[:], mt_sb[:, icb, cb * 128:(cb + 1) * 128],
                        praw5[icb][:], start=(icb == 0), stop=(icb == 3))
                mp5_ps.append(mp)
            sim_ps = ptiny.tile([1, K2], F32, tag="pt")
            for cb in range(2):
                e = small.tile([128, K2], F32, tag=f"esim_{cb}")
                nc.vector.tensor_mul(e[:], praw4[cb][:], mp5_ps[cb][:])
                nc.tensor.matmul(sim_ps[:], ones_sb[:], e[:],
                                 start=(cb == 0), stop=(cb == 1))
            gated = small.tile([1, K2], F32, tag="gated")
            nc.vector.tensor_mul(gated[:], sim_ps[:], sg_sb[:])
            h_ps = ptiny.tile([HID, K2], F32, tag="pt")
            nc.tensor.matmul(h_ps[:], w1_sb[:], gated[:])
            h_sb = small.tile([HID, K2], F32, tag="h")
            nc.scalar.activation(h_sb[:], h_ps[:], ACTF.Relu, bias=b1_sb[:])
            lg_ps = ptiny.tile([K2, K2], F32, tag="pt")
            nc.tensor.matmul(lg_ps[:], h_sb[:], w2_sb[:])
            lg2 = small.tile([K2, K2], F32, tag="lg2")
            nc.vector.tensor_add(lg2[:], lg_ps[:], b2_sb[:])
            nmx = small.tile([K2, 1], F32, tag="nmx")
            nc.vector.tensor_reduce(nmx[:], lg2[:], AX.X, ALU.max, negate=True)
            esb = small.tile([K2, K2], F32, tag="esb")
            esum = small.tile([K2, 1], F32, tag="esum")
            nc.scalar.activation(esb[:], lg2[:], ACTF.Exp, bias=nmx[:],
                                 accum_out=esum[:])
            rs = small.tile([K2, 1], F32, tag="rs")
            nc.vector.reciprocal(rs[:], esum[:])
            kern = small.tile([K2, K2], F32, tag="kern")
            nc.vector.tensor_scalar_mul(kern[:], esb[:], rs[:])
            kd = dram.tile([K2, K2], F32, tag="kdram")
            dma(kd[:], kern[:])
            kbc = wts.tile([128, 81], F32, tag="kbc")
            dma(kbc[:], kd[:].rearrange("a b -> (a b)")[None, :].broadcast_to([128, 81]))

            # ---- BN stats -> AllReduce -> scale/bias -> X = silu -----
            stats = small.tile([128, 4], F32, tag="stats")
            for cb in range(2):
                nc.vector.tensor_reduce(stats[:, cb:cb + 1], ysum_p[cb][:],
                                        AX.X, ALU.add)
                nc.vector.tensor_reduce(stats[:, 2 + cb:3 + cb], ysq_p[cb][:],
                                        AX.X, ALU.add)
            stin = dram.tile([128, 4], F32, tag="stin")
            stout = dram.tile([128, 4], F32, tag="stout")
            dma(stin[:], stats[:])
            nc.gpsimd.collective_compute(
                "AllReduce", ALU.add,
                replica_groups=[list(range(NCORES))],
                ins=[stin.opt()], outs=[stout.opt()])
            stats2 = small.tile([128, 4], F32, tag="stats2")
            dma(stats2[:], stout[:])

            xp = []
            s_t, b_t = [], []
            for cb in range(2):
                mu = small.tile([128, 1], F32, tag=f"mu{cb}")
                nc.vector.tensor_scalar_mul(mu[:], stats2[:, cb:cb + 1], 1.0 / NSTAT)
                musq = small.tile([128, 1], F32, tag=f"musq{cb}")
                nc.vector.tensor_mul(musq[:], mu[:], mu[:])
                # musq - EPS, so that sq/N - (musq - EPS) = var + EPS
                nc.vector.tensor_scalar_add(musq[:], musq[:], -EPS)
                var = small.tile([128, 1], F32, tag=f"var{cb}")
                nc.vector.scalar_tensor_tensor(
                    out=var[:], in0=stats2[:, 2 + cb:3 + cb], scalar=1.0 / NSTAT,
                    in1=musq[:], op0=ALU.mult, op1=ALU.subtract)
                sd = small.tile([128, 1], F32, tag=f"sd{cb}")
                nc.scalar.activation(sd[:], var[:], ACTF.Sqrt)
                rinv = small.tile([128, 1], F32, tag=f"rinv{cb}")
                nc.vector.reciprocal(rinv[:], sd[:])
                st = small.tile([128, 1], F32, tag=f"sbn{cb}")
                nc.vector.tensor_mul(st[:], gam_sb[cb][:], rinv[:])
                t1 = small.tile([128, 1], F32, tag=f"t1{cb}")
                nc.vector.tensor_scalar_mul(t1[:], mu[:], st[:])
                bt = small.tile([128, 1], F32, tag=f"bbn{cb}")
                nc.vector.tensor_sub(bt[:], bet_sb[cb][:], t1[:])
                s_t.append(st)
                b_t.append(bt)
            for cb in range(2):
                x = pad.tile([128, 66, 66], F32, tag="pad66")
                dma(x[:, 0, :].bitcast(F32R),
                    zpd[None, :].broadcast_to([128, 66]).bitcast(F32R))
                dma(x[:, 65, :].bitcast(F32R),
                    zpd[None, :].broadcast_to([128, 66]).bitcast(F32R))
                dma(x[:, 1:65, 0:1].bitcast(F32R),
                    zpd[None, :64, None].broadcast_to([128, 64, 1]).bitcast(F32R))
                dma(x[:, 1:65, 65:66].bitcast(F32R),
                    zpd[None, :64, None].broadcast_to([128, 64, 1]).bitcast(F32R))
                nc.scalar.activation(
                    x[:, 1:65, 1:65].bitcast(F32R),
                    y_sb[cb][:].rearrange("p (h w) -> p h w", h=H),
                    ACTF.Silu, bias=b_t[cb][:], scale=s_t[cb][:])
                xp.append(x)

            # ---- fused_red = proj(reshape(fused)) --------------------
            t_sb = big.tile([128, NPIX], F32, tag="mid16")
            for pt in range(8):
                ps = pmain.tile([128, 512], F32, tag="ps")
                for icb in range(2):
                    nc.tensor.matmul(
                        ps[:], wrs_sb[:, icb, :].bitcast(F32R),
                        fused[icb][:, pt * 512:(pt + 1) * 512].bitcast(F32R),
                        start=(icb == 0), stop=(icb == 1))
                nc.scalar.copy(t_sb[:, pt * 512:(pt + 1) * 512].bitcast(F32R), ps[:])
            fr = []
            for cb in range(2):
                f = big.tile([128, NPIX], F32, tag="mid16")
                for pt in range(8):
                    ps = pmain.tile([128, 512], F32, tag="ps")
                    nc.tensor.matmul(
                        ps[:], wpr_sb[:, cb * 128:(cb + 1) * 128].bitcast(F32R),
                        t_sb[:, pt * 512:(pt + 1) * 512].bitcast(F32R))
                    nc.scalar.copy(f[:, pt * 512:(pt + 1) * 512], ps[:])
                fr.append(f)

            # ---- dynamic filter + final add --------------------------
            # fp32r matmuls need even free sizes, so every region is computed
            # as a uniform 22x22 window; odd-sized bands (21) overlap one row/
            # col into the neighbor band (with this region's weights) and the
            # final add consumes only the correct sub-rectangle.
            # (use_start, use_len, comp_start, psum_off) per band:
            DBANDS = [(0, 22, 0, 0), (22, 21, 22, 0), (43, 21, 42, 1)]
            for ry, (r0, nr, gr, orow) in enumerate(DBANDS):
                for rx, (c0, ncc, gc, ocol) in enumerate(DBANDS):
                    reg = ry * 3 + rx
                    pds = [pdyn.tile([128, 484], F32, tag="pd",
                                     name=f"pd{reg}_{i}") for i in range(2)]
                    for tap in range(9):
                        dy, dx = tap // 3, tap % 3
                        rk = reg * 9 + tap
                        idt = idp.tile([128, 128], F32, tag="idt")
                        if tap % 2 == 0:
                            nc.vector.tensor_scalar_mul(idt[:].bitcast(F32R),
                                                        eye_sb[:], kbc[:, rk:rk + 1])
                        else:
                            nc.scalar.mul(idt[:].bitcast(F32R), eye_sb[:],
                                          kbc[:, rk:rk + 1])
                        for cb in range(2):
                            nc.tensor.matmul(
                                pds[cb][:], idt[:].bitcast(F32R),
                                xp[cb][:, gr + dy:gr + dy + 22,
                                       gc + dx:gc + dx + 22].bitcast(F32R),
                                start=(tap == 0), stop=(tap == 8))
                    for cb in range(2):
                        fv = fr[cb][:].rearrange("p (h w) -> p h w", h=H)
                        pv = pds[cb][:].rearrange("p (a b) -> p a b", a=22)
                        nc.vector.tensor_add(
                            fv[:, r0:r0 + nr, c0:c0 + ncc],
                            pv[:, orow:orow + nr, ocol:ocol + ncc],
                            fv[:, r0:r0 + nr, c0:c0 + ncc])

            # ---- out -------------------------------------------------
            for cb in range(2):
                dma(outd[cb * 128:(cb + 1) * 128], fr[cb][:])

    nc.compile()
    return nc


def _prep_inputs(inputs):
    """Host-side parameter folding + per-core input maps."""
    f = np.float32
    c4 = np.ascontiguousarray(inputs["c4"], f).reshape(B, C4, NPIX)
    c5 = np.ascontiguousarray(inputs["c5"], f).reshape(B, C5, 1024)
    wc4 = np.ascontiguousarray(
        np.transpose(np.asarray(inputs["w_c4_proc"], f).reshape(OC, C4, 9),
                     (1, 2, 0)))                      # (ic, tap, oc)
    wc1 = np.ascontiguousarray(np.asarray(inputs["w_conv1"], f).reshape(OC, C5).T)
    wtf = np.ascontiguousarray(np.asarray(inputs["w_to_fuse"], f).reshape(OC, C4).T)
    wrs = np.ascontiguousarray(np.asarray(inputs["w_reshape"], f).reshape(FR, C4).T)
    wpr = np.ascontiguousarray(np.asarray(inputs["w_proj"], f).reshape(OC, FR).T)
    w4 = np.asarray(inputs["w_sim4"], f).reshape(64, C4)
    w5 = np.asarray(inputs["w_sim5"], f).reshape(64, C5)
    mt = np.ascontiguousarray(w5.T @ w4)              # (c5, c4) = (W4^T W5)^T
    sig = 1.0 / (1.0 + np.exp(-np.asarray(inputs["mask_raw"], np.float64)))
    fac = np.array([P5FAC[i] * P5FAC[j] for i in range(3) for j in range(3)],
                   np.float64)
    sgp = (sig * fac / (484.0 * 484.0)).astype(f)
    maps = []
    shared = dict(
        wc4t=wc4, wc1t=wc1, wtft=wtf, wrst=wrs, wprt=wpr, mt=mt,
        w1=np.ascontiguousarray(np.asarray(inputs["kg_w1"], f).reshape(HID)),
        b1=np.ascontiguousarray(np.asarray(inputs["kg_b1"], f)),
        w2t=np.ascontiguousarray(np.asarray(inputs["kg_w2"], f).T),
        b2t=np.ascontiguousarray(np.tile(np.asarray(inputs["kg_b2"], f), (K2, 1))),
        sgp=sgp,
        gam=np.ascontiguousarray(np.asarray(inputs["bn_gamma"], f)),
        bet=np.ascontiguousarray(np.asarray(inputs["bn_beta"], f)),
        i128=np.eye(128, dtype=f),
        zpad=np.zeros(66, dtype=f),
    )
    for b in range(B):
        m = dict(shared)
        m["c4"] = np.ascontiguousarray(c4[b])
        m["c5"] = np.ascontiguousarray(c5[b])
        maps.append(m)
    return maps


def _run(inputs, trace=False):
    if "nc" not in _CACHE:
        _CACHE["nc"] = _build()
    nc = _CACHE["nc"]
    maps = _prep_inputs(inputs)
    return run_bass_kernel_spmd(nc, maps, list(range(NCORES)), trace=trace)


def kernel(**inputs) -> np.ndarray:
    res = _run(inputs, trace=False)
    out = np.stack([res.results[i]["o_out"] for i in range(NCORES)])
    return out.reshape(B, OC, H, W).astype(np.float32)


# revision 8
# speedup vs baseline: 5.5461x; 5.5461x over previous
"""Trainium2 Bass kernel for nn_CSDKM_66417374265458 (dense_cnn).

Data-parallel over batch B=8 across 8 NeuronCores (one image per core, all
parameters replicated). The only cross-core communication is a 2KB AllReduce
of the BatchNorm batch statistics (sum / sum-of-squares per channel).

Per-core pipeline (all shapes per batch element):
  c4 (256,64,64), c5 (512,32,32)
  c4_proc = conv3x3(c4)                  -> shifted-window matmuls on PE
  c5_proc = conv1x1(c5) at 32x32, then nearest-upsample (1x1 conv commutes
            with nearest upsampling)
  fused   = c4_proc + up(c5_proc)        -> fused into the PSUM->SBUF move
  y       = conv1x1(fused); BN batch stats -> AllReduce -> X = silu(s*y+b)
  sim/gate path: adaptive pools as rectangle reductions, w_sim4^T w_sim5
            folded on host, tiny matmuls + softmax -> per-region 3x3 kernels
  dynfilter: out = sum_k kern[region,k] * shift_k(X) -> scaled-identity
            matmuls on PE accumulating in PSUM (region-rect free APs)
  out     = dynfilter(X) + conv1x1(conv1x1(fused))
"""
import sys

sys.path.insert(0, "/opt/trn_rl_repo")

import numpy as np

import concourse.bass as bass  # noqa: F401  (engine types referenced via nc)
import concourse.bacc as bacc
import concourse.tile as tile
from concourse import mybir
from concourse.bass_utils import run_bass_kernel_spmd

F32 = mybir.dt.float32
F32R = mybir.dt.float32r
ALU = mybir.AluOpType
ACTF = mybir.ActivationFunctionType
AX = mybir.AxisListType

B, C4, C5, H, W = 8, 256, 512, 64, 64
OC, FR, HID = 256, 128, 16
S, K2 = 3, 9
EPS = 1e-5
NCORES = 8
NPIX = H * W  # 4096
NSTAT = float(B * NPIX)  # BN sample count per channel

# Output-space region bands (start, len) for rows and cols: pidx regions.
BANDS = [(0, 22), (22, 21), (43, 21)]
# pool4 bins on the 64x64 grid (overlapping 22-wide intervals).
P4B = [(0, 22), (21, 22), (42, 22)]
# pool5 on the 32x32 grid: the upsampled 22-wide bin maps to interval sums
# over c5 rows; bin i = sum over listed (start, count) intervals, and a
# host-folded factor (uniform bins count each row twice).
P5IV = {0: [(0, 11)], 1: [(10, 12), (11, 10)], 2: [(21, 11)]}
P5FAC = {0: 2.0, 1: 1.0, 2: 2.0}

_CACHE = {}


def _build():
    nc = bacc.Bacc("TRN2", target_bir_lowering=False, debug=False,
                   num_devices=NCORES)

    # ---- DRAM I/O -------------------------------------------------------
    c4d = nc.dram_tensor("c4", [C4, 66 * 66], F32, kind="ExternalInput").ap()
    c5d = nc.dram_tensor("c5", [C5, 1024], F32, kind="ExternalInput").ap()
    wc4d = nc.dram_tensor("wc4t", [C4, 9, OC], F32, kind="ExternalInput").ap()
    wc1d = nc.dram_tensor("wc1t", [C5, OC], F32, kind="ExternalInput").ap()
    wtfd = nc.dram_tensor("wtft", [C4, OC], F32, kind="ExternalInput").ap()
    wrsd = nc.dram_tensor("wrst", [C4, FR], F32, kind="ExternalInput").ap()
    wprd = nc.dram_tensor("wprt", [FR, OC], F32, kind="ExternalInput").ap()
    mtd = nc.dram_tensor("mt", [C5, C4], F32, kind="ExternalInput").ap()
    w1d = nc.dram_tensor("w1", [HID], F32, kind="ExternalInput").ap()
    b1d = nc.dram_tensor("b1", [HID], F32, kind="ExternalInput").ap()
    w2d = nc.dram_tensor("w2t", [HID, K2], F32, kind="ExternalInput").ap()
    b2d = nc.dram_tensor("b2t", [K2, K2], F32, kind="ExternalInput").ap()
    sgd = nc.dram_tensor("sgp", [K2], F32, kind="ExternalInput").ap()
    gmd = nc.dram_tensor("gam", [OC], F32, kind="ExternalInput").ap()
    btd = nc.dram_tensor("bet", [OC], F32, kind="ExternalInput").ap()
    eyd = nc.dram_tensor("i128", [128, 128], F32, kind="ExternalInput").ap()
    outd = nc.dram_tensor("o_out", [OC, NPIX], F32, kind="ExternalOutput").ap()

    with tile.TileContext(nc) as tc:
        with (
            tc.tile_pool(name="big", bufs=3) as big,
            tc.tile_pool(name="ypool", bufs=2) as ypool,
            tc.tile_pool(name="pad", bufs=2) as pad,
            tc.tile_pool(name="c5pool", bufs=6) as c5pool,
            tc.tile_pool(name="wts", bufs=1) as wts,
            tc.tile_pool(name="small", bufs=1) as small,
            tc.tile_pool(name="scr", bufs=2) as scr,
            tc.tile_pool(name="idp", bufs=6) as idp,
            tc.tile_pool(name="pmain", bufs=3, space="PSUM") as pmain,
            tc.tile_pool(name="pdyn", bufs=3, space="PSUM") as pdyn,
            tc.tile_pool(name="ptiny", bufs=2, space="PSUM") as ptiny,
            tc.tile_pool(name="dram", bufs=1, space="DRAM") as dram,
        ):
            dma = nc.sync.dma_start

            # ---- weights / consts in --------------------------------
            wc4_sb = []
            for icb in range(2):
                t = wts.tile([128, 9, OC], F32, tag=f"wc4_{icb}")
                dma(t[:].rearrange("p a b -> p (a b)").bitcast(F32R),
                    wc4d[icb * 128:(icb + 1) * 128].rearrange("p a b -> p (a b)").bitcast(F32R))
                wc4_sb.append(t)
            wc1_sb = wts.tile([128, 4, OC], F32, tag="wc1")
            dma(wc1_sb[:].bitcast(F32R), wc1d.rearrange("(b p) o -> p b o", p=128).bitcast(F32R))
            wtf_sb = wts.tile([128, 2, OC], F32, tag="wtf")
            dma(wtf_sb[:].bitcast(F32R), wtfd.rearrange("(b p) o -> p b o", p=128).bitcast(F32R))
            wrs_sb = wts.tile([128, 2, FR], F32, tag="wrs")
            dma(wrs_sb[:].bitcast(F32R), wrsd.rearrange("(b p) o -> p b o", p=128).bitcast(F32R))
            wpr_sb = wts.tile([128, OC], F32, tag="wpr")
            dma(wpr_sb[:].bitcast(F32R), wprd.bitcast(F32R))
            mt_sb = wts.tile([128, 4, C4], F32, tag="mt")
            dma(mt_sb[:], mtd.rearrange("(b p) o -> p b o", p=128))
            eye_sb = wts.tile([128, 128], F32, tag="eye")
            dma(eye_sb[:], eyd)
            w1_sb = wts.tile([1, HID], F32, tag="w1")
            dma(w1_sb[:], w1d[None, :])
            b1_sb = wts.tile([HID, 1], F32, tag="b1")
            dma(b1_sb[:], b1d[:, None])
            w2_sb = wts.tile([HID, K2], F32, tag="w2")
            dma(w2_sb[:], w2d)
            b2_sb = wts.tile([K2, K2], F32, tag="b2")
            dma(b2_sb[:], b2d)
            sg_sb = wts.tile([1, K2], F32, tag="sg")
            dma(sg_sb[:], sgd[None, :])
            gam_sb, bet_sb = [], []
            for cb in range(2):
                g = wts.tile([128, 1], F32, tag=f"gam{cb}")
                dma(g[:], gmd[cb * 128:(cb + 1) * 128][:, None])
                gam_sb.append(g)
                bt = wts.tile([128, 1], F32, tag=f"bet{cb}")
                dma(bt[:], btd[cb * 128:(cb + 1) * 128][:, None])
                bet_sb.append(bt)
            ones_sb = wts.tile([128, 1], F32, tag="ones")
            nc.vector.memset(ones_sb[:], 1.0)

            # ---- data in: c4 pre-padded on host (contiguous), c5 plain
            zrow = wts.tile([128, 66], F32, tag="zrow")
            nc.vector.memset(zrow[:], 0.0)
            c4p = []
            for cb in range(2):
                t = pad.tile([128, 66, 66], F32, tag="pad66")
                dma(t[:].rearrange("p a b -> p (a b)").bitcast(F32R),
                    c4d[cb * 128:(cb + 1) * 128].bitcast(F32R))
                c4p.append(t)
            c5_sb = []
            for icb in range(4):
                t = c5pool.tile([128, 1024], F32, tag="c5in", bufs=4)
                dma(t[:].bitcast(F32R), c5d[icb * 128:(icb + 1) * 128].bitcast(F32R))
                c5_sb.append(t)

            # ---- pool4: 9 overlapping 22x22 rect sums per ch block ---
            praw4 = []
            for cb in range(2):
                p4 = small.tile([128, K2], F32, tag=f"praw4_{cb}")
                for i, (r0, nr) in enumerate(P4B):
                    for j, (c0, ncc) in enumerate(P4B):
                        nc.vector.tensor_reduce(
                            p4[:, i * 3 + j: i * 3 + j + 1],
                            c4p[cb][:, r0 + 1:r0 + 1 + nr, c0 + 1:c0 + 1 + ncc],
                            AX.XY, ALU.add)
                praw4.append(p4)

            # ---- pool5: separable interval sums on the 32x32 grid ----
            praw5 = []
            for icb in range(4):
                v = c5_sb[icb][:].rearrange("p (h w) -> p h w", h=32)
                cs = small.tile([128, 3, 32], F32, tag=f"cs_{icb}")
                for j in range(3):
                    ivs = P5IV[j]
                    nc.vector.tensor_reduce(
                        cs[:, j, :][:, :, None], v[:, :, ivs[0][0]:ivs[0][0] + ivs[0][1]],
                        AX.X, ALU.add)
                    if len(ivs) > 1:
                        tmp = small.tile([128, 32], F32, tag=f"cstmp_{icb}")
                        nc.vector.tensor_reduce(
                            tmp[:, :, None], v[:, :, ivs[1][0]:ivs[1][0] + ivs[1][1]],
                            AX.X, ALU.add)
                        nc.vector.tensor_add(cs[:, j, :], cs[:, j, :], tmp[:])
                p5 = small.tile([128, K2], F32, tag=f"praw5_{icb}")
                for i in range(3):
                    ivs = P5IV[i]
                    for j in range(3):
                        sl = p5[:, i * 3 + j: i * 3 + j + 1]
                        nc.vector.tensor_reduce(
                            sl, cs[:, j, ivs[0][0]:ivs[0][0] + ivs[0][1]],
                            AX.X, ALU.add)
                        if len(ivs) > 1:
                            t1 = small.tile([128, 1], F32, tag=f"p5tmp_{icb}")
                            nc.vector.tensor_reduce(
                                t1[:], cs[:, j, ivs[1][0]:ivs[1][0] + ivs[1][1]],
                                AX.X, ALU.add)
                            nc.vector.tensor_add(sl, sl, t1[:])
                praw5.append(p5)

            # ---- c5 1x1 conv at 32x32 -------------------------------
            c5p_sb = []
            for cb in range(2):
                t = c5pool.tile([128, 1024], F32, tag="c5p", bufs=2)
                for pt in range(2):
                    ps = pmain.tile([128, 512], F32, tag="ps")
                    for icb in range(4):
                        nc.tensor.matmul(
                            ps[:],
                            wc1_sb[:, icb, cb * 128:(cb + 1) * 128].bitcast(F32R),
                            c5_sb[icb][:, pt * 512:(pt + 1) * 512].bitcast(F32R),
                            start=(icb == 0), stop=(icb == 3))
                    nc.scalar.copy(t[:, pt * 512:(pt + 1) * 512], ps[:])
                c5p_sb.append(t)

            # ---- materialize nearest-upsampled c5_proc ---------------
            # c5up shares slots with the later y tiles (disjoint lifetimes).
            c5up = []
            for cb in range(2):
                u = ypool.tile([128, NPIX], F32, tag="y", name=f"c5up{cb}")
                uv = u[:].rearrange("p (h a w b) -> p h a w b", h=32, a=2, b=2)
                sv = c5p_sb[cb][:].rearrange("p (h w) -> p h w", h=32)
                for a in range(2):
                    for bb in range(2):
                        nc.vector.tensor_copy(uv[:, :, a, :, bb], sv)
                c5up.append(u)

            # ---- conv3x3 + fused assembly (+ upsampled c5_proc) ------
            fused = []
            for cb in range(2):
                f = big.tile([128, NPIX], F32, tag="mid16")
                for pt in range(8):
                    ps = pmain.tile([128, 512], F32, tag="ps")
                    for icb in range(2):
                        for tap in range(9):
                            dy, dx = tap // 3, tap % 3
                            nc.tensor.matmul(
                                ps[:],
                                wc4_sb[icb][:, tap, cb * 128:(cb + 1) * 128].bitcast(F32R),
                                c4p[icb][:, pt * 8 + dy:pt * 8 + dy + 8, dx:dx + 64].bitcast(F32R),
                                start=(icb == 0 and tap == 0),
                                stop=(icb == 1 and tap == 8))
                    nc.vector.scalar_tensor_tensor(
                        out=f[:, pt * 512:(pt + 1) * 512].bitcast(F32R),
                        in0=ps[:],
                        scalar=1.0, in1=c5up[cb][:, pt * 512:(pt + 1) * 512],
                        op0=ALU.mult, op1=ALU.add)
                fused.append(f)

            # ---- to_fuse 1x1 conv -> y (+ stats partials) ------------
            y_sb, ysum_p, ysq_p = [], [], []
            for cb in range(2):
                y = ypool.tile([128, NPIX], F32, tag="y")
                su = small.tile([128, 8], F32, tag=f"ysum_{cb}")
                sq = small.tile([128, 8], F32, tag=f"ysq_{cb}")
                for pt in range(8):
                    ps = pmain.tile([128, 512], F32, tag="ps")
                    for icb in range(2):
                        nc.tensor.matmul(
                            ps[:],
                            wtf_sb[:, icb, cb * 128:(cb + 1) * 128].bitcast(F32R),
                            fused[icb][:, pt * 512:(pt + 1) * 512].bitcast(F32R),
                            start=(icb == 0), stop=(icb == 1))
                    nc.scalar.activation(
                        y[:, pt * 512:(pt + 1) * 512], ps[:], ACTF.Copy,
                        accum_out=su[:, pt:pt + 1])
                    sc = scr.tile([128, 512], F32, tag="sqscr")
                    nc.scalar.activation(sc[:], ps[:], ACTF.Square,
                                         accum_out=sq[:, pt:pt + 1])
                y_sb.append(y)
                ysum_p.append(su)
                ysq_p.append(sq)

            # ---- sim / gating / per-region kernels -------------------
            mp5_ps = []
            for cb in range(2):
                mp = ptiny.tile([128, K2], F32, tag="pt")
                for icb in range(4):
                    nc.tensor.matmul(
                        mp[:], mt_sb[:, icb, cb * 128:(cb + 1) * 128],
                        praw5[icb][:], start=(icb == 0), stop=(icb == 3))
                mp5_ps.append(mp)
            sim_ps = ptiny.tile([1, K2], F32, tag="pt")
            for cb in range(2):
                e = small.tile([128, K2], F32, tag=f"esim_{cb}")
                nc.vector.tensor_mul(e[:], praw4[cb][:], mp5_ps[cb][:])
                nc.tensor.matmul(sim_ps[:], ones_sb[:], e[:],
                                 start=(cb == 0), stop=(cb == 1))
            gated = small.tile([1, K2], F32, tag="gated")
            nc.vector.tensor_mul(gated[:], sim_ps[:], sg_sb[:])
            h_ps = ptiny.tile([HID, K2], F32, tag="pt")
            nc.tensor.matmul(h_ps[:], w1_sb[:], gated[:])
            h_sb = small.tile([HID, K2], F32, tag="h")
            nc.scalar.activation(h_sb[:], h_ps[:], ACTF.Relu, bias=b1_sb[:])
            lg_ps = ptiny.tile([K2, K2], F32, tag="pt")
            nc.tensor.matmul(lg_ps[:], h_sb[:], w2_sb[:])
            lg2 = small.tile([K2, K2], F32, tag="lg2")
            nc.vector.tensor_add(lg2[:], lg_ps[:], b2_sb[:])
            nmx = small.tile([K2, 1], F32, tag="nmx")
            nc.vector.tensor_reduce(nmx[:], lg2[:], AX.X, ALU.max, negate=True)
            esb = small.tile([K2, K2], F32, tag="esb")
            esum = small.tile([K2, 1], F32, tag="esum")
            nc.scalar.activation(esb[:], lg2[:], ACTF.Exp, bias=nmx[:],
                                 accum_out=esum[:])
            rs = small.tile([K2, 1], F32, tag="rs")
            nc.vector.reciprocal(rs[:], esum[:])
            kern = small.tile([K2, K2], F32, tag="kern")
            nc.vector.tensor_scalar_mul(kern[:], esb[:], rs[:])
            kd = dram.tile([K2, K2], F32, tag="kdram")
            dma(kd[:], kern[:])
            kbc = wts.tile([128, 81], F32, tag="kbc")
            dma(kbc[:], kd[:].rearrange("a b -> (a b)")[None, :].broadcast_to([128, 81]))

            # ---- BN stats -> AllReduce -> scale/bias -> X = silu -----
            stats = small.tile([128, 4], F32, tag="stats")
            for cb in range(2):
                nc.vector.tensor_reduce(stats[:, cb:cb + 1], ysum_p[cb][:],
                                        AX.X, ALU.add)
                nc.vector.tensor_reduce(stats[:, 2 + cb:3 + cb], ysq_p[cb][:],
                                        AX.X, ALU.add)
            stin = dram.tile([128, 4], F32, tag="stin")
            stout = dram.tile([128, 4], F32, tag="stout")
            dma(stin[:], stats[:])
            nc.gpsimd.collective_compute(
                "AllReduce", ALU.add,
                replica_groups=[list(range(NCORES))],
                ins=[stin.opt()], outs=[stout.opt()])
            stats2 = small.tile([128, 4], F32, tag="stats2")
            dma(stats2[:], stout[:])

            xp = []
            s_t, b_t = [], []
            for cb in range(2):
                mu = small.tile([128, 1], F32, tag=f"mu{cb}")
                nc.vector.tensor_scalar_mul(mu[:], stats2[:, cb:cb + 1], 1.0 / NSTAT)
                musq = small.tile([128, 1], F32, tag=f"musq{cb}")
                nc.vector.tensor_mul(musq[:], mu[:], mu[:])
                # musq - EPS, so that sq/N - (musq - EPS) = var + EPS
                nc.vector.tensor_scalar_add(musq[:], musq[:], -EPS)
                var = small.tile([128, 1], F32, tag=f"var{cb}")
                nc.vector.scalar_tensor_tensor(
                    out=var[:], in0=stats2[:, 2 + cb:3 + cb], scalar=1.0 / NSTAT,
                    in1=musq[:], op0=ALU.mult, op1=ALU.subtract)
                sd = small.tile([128, 1], F32, tag=f"sd{cb}")
                nc.scalar.activation(sd[:], var[:], ACTF.Sqrt)
                rinv = small.tile([128, 1], F32, tag=f"rinv{cb}")
                nc.vector.reciprocal(rinv[:], sd[:])
                st = small.tile([128, 1], F32, tag=f"sbn{cb}")
                nc.vector.tensor_mul(st[:], gam_sb[cb][:], rinv[:])
                t1 = small.tile([128, 1], F32, tag=f"t1{cb}")
                nc.vector.tensor_scalar_mul(t1[:], mu[:], st[:])
                bt = small.tile([128, 1], F32, tag=f"bbn{cb}")
                nc.vector.tensor_sub(bt[:], bet_sb[cb][:], t1[:])
                s_t.append(st)
                b_t.append(bt)
            for cb in range(2):
                x = pad.tile([128, 66, 66], F32, tag="pad66")
                nc.vector.tensor_copy(x[:, 0, :].bitcast(F32R), zrow[:])
                nc.vector.tensor_copy(x[:, 65, :].bitcast(F32R), zrow[:])
                xs = x[:].rearrange("p a b -> p (a b)")[:, 65:65 + 65 * 66]
                nc.vector.tensor_copy(
                    xs.rearrange("p (r t) -> p r t", t=66)[:, :, 0:2].bitcast(F32R),
                    zrow[:, None, 0:2].broadcast_to([128, 65, 2]))
                nc.scalar.activation(
                    x[:, 1:65, 1:65].bitcast(F32R),
                    y_sb[cb][:].rearrange("p (h w) -> p h w", h=H),
                    ACTF.Silu, bias=b_t[cb][:], scale=s_t[cb][:])
                xp.append(x)

            # ---- fused_red = proj(reshape(fused)) --------------------
            t_sb = big.tile([128, NPIX], F32, tag="mid16")
            for pt in range(8):
                ps = pmain.tile([128, 512], F32, tag="ps")
                for icb in range(2):
                    nc.tensor.matmul(
                        ps[:], wrs_sb[:, icb, :].bitcast(F32R),
                        fused[icb][:, pt * 512:(pt + 1) * 512].bitcast(F32R),
                        start=(icb == 0), stop=(icb == 1))
                nc.scalar.copy(t_sb[:, pt * 512:(pt + 1) * 512].bitcast(F32R), ps[:])
            fr = []
            for cb in range(2):
                f = big.tile([128, NPIX], F32, tag="mid16")
                for pt in range(8):
                    ps = pmain.tile([128, 512], F32, tag="ps")
                    nc.tensor.matmul(
                        ps[:], wpr_sb[:, cb * 128:(cb + 1) * 128].bitcast(F32R),
                        t_sb[:, pt * 512:(pt + 1) * 512].bitcast(F32R))
                    nc.scalar.copy(f[:, pt * 512:(pt + 1) * 512], ps[:])
                fr.append(f)

            # ---- dynamic filter + final add --------------------------
            # fp32r matmuls need even free sizes, so every region is computed
            # as a uniform 22x22 window; odd-sized bands (21) overlap one row/
            # col into the neighbor band (with this region's weights) and the
            # final add consumes only the correct sub-rectangle.
            # (use_start, use_len, comp_start, psum_off) per band:
            DBANDS = [(0, 22, 0, 0), (22, 21, 22, 0), (43, 21, 42, 1)]
            for ry, (r0, nr, gr, orow) in enumerate(DBANDS):
                for rx, (c0, ncc, gc, ocol) in enumerate(DBANDS):
                    reg = ry * 3 + rx
                    pds = [pdyn.tile([128, 484], F32, tag="pd",
                                     name=f"pd{reg}_{i}") for i in range(2)]
                    for tap in range(9):
                        dy, dx = tap // 3, tap % 3
                        rk = reg * 9 + tap
                        idt = idp.tile([128, 128], F32, tag="idt")
                        if tap % 2 == 0:
                            nc.vector.tensor_scalar_mul(idt[:].bitcast(F32R),
                                                        eye_sb[:], kbc[:, rk:rk + 1])
                        else:
                            nc.scalar.mul(idt[:].bitcast(F32R), eye_sb[:],
                                          kbc[:, rk:rk + 1])
                        for cb in range(2):
                            nc.tensor.matmul(
                                pds[cb][:], idt[:].bitcast(F32R),
                                xp[cb][:, gr + dy:gr + dy + 22,
                                       gc + dx:gc + dx + 22].bitcast(F32R),
                                start=(tap == 0), stop=(tap == 8))
                    for cb in range(2):
                        fv = fr[cb][:].rearrange("p (h w) -> p h w", h=H)
                        pv = pds[cb][:].rearrange("p (a b) -> p a b", a=22)
                        nc.vector.tensor_add(
                            fv[:, r0:r0 + nr, c0:c0 + ncc],
                            pv[:, orow:orow + nr, ocol:ocol + ncc],
                            fv[:, r0:r0 + nr, c0:c0 + ncc])

            # ---- out -------------------------------------------------
            for cb in range(2):
                dma(outd[cb * 128:(cb + 1) * 128], fr[cb][:])

    nc.compile()
    return nc


def _prep_inputs(inputs):
    """Host-side parameter folding + per-core input maps."""
    f = np.float32
    c4r = np.asarray(inputs["c4"], f).reshape(B, C4, H, W)
    c4 = np.zeros((B, C4, 66, 66), f)
    c4[:, :, 1:65, 1:65] = c4r
    c4 = c4.reshape(B, C4, 66 * 66)
    c5 = np.ascontiguousarray(inputs["c5"], f).reshape(B, C5, 1024)
    wc4 = np.ascontiguousarray(
        np.transpose(np.asarray(inputs["w_c4_proc"], f).reshape(OC, C4, 9),
                     (1, 2, 0)))                      # (ic, tap, oc)
    wc1 = np.ascontiguousarray(np.asarray(inputs["w_conv1"], f).reshape(OC, C5).T)
    wtf = np.ascontiguousarray(np.asarray(inputs["w_to_fuse"], f).reshape(OC, C4).T)
    wrs = np.ascontiguousarray(np.asarray(inputs["w_reshape"], f).reshape(FR, C4).T)
    wpr = np.ascontiguousarray(np.asarray(inputs["w_proj"], f).reshape(OC, FR).T)
    w4 = np.asarray(inputs["w_sim4"], f).reshape(64, C4)
    w5 = np.asarray(inputs["w_sim5"], f).reshape(64, C5)
    mt = np.ascontiguousarray(w5.T @ w4)              # (c5, c4) = (W4^T W5)^T
    sig = 1.0 / (1.0 + np.exp(-np.asarray(inputs["mask_raw"], np.float64)))
    fac = np.array([P5FAC[i] * P5FAC[j] for i in range(3) for j in range(3)],
                   np.float64)
    sgp = (sig * fac / (484.0 * 484.0)).astype(f)
    maps = []
    shared = dict(
        wc4t=wc4, wc1t=wc1, wtft=wtf, wrst=wrs, wprt=wpr, mt=mt,
        w1=np.ascontiguousarray(np.asarray(inputs["kg_w1"], f).reshape(HID)),
        b1=np.ascontiguousarray(np.asarray(inputs["kg_b1"], f)),
        w2t=np.ascontiguousarray(np.asarray(inputs["kg_w2"], f).T),
        b2t=np.ascontiguousarray(np.tile(np.asarray(inputs["kg_b2"], f), (K2, 1))),
        sgp=sgp,
        gam=np.ascontiguousarray(np.asarray(inputs["bn_gamma"], f)),
        bet=np.ascontiguousarray(np.asarray(inputs["bn_beta"], f)),
        i128=np.eye(128, dtype=f),
    )
    for b in range(B):
        m = dict(shared)
        m["c4"] = np.ascontiguousarray(c4[b])
        m["c5"] = np.ascontiguousarray(c5[b])
        maps.append(m)
    return maps


def _run(inputs, trace=False):
    if "nc" not in _CACHE:
        _CACHE["nc"] = _build()
    nc = _CACHE["nc"]
    maps = _prep_inputs(inputs)
    return run_bass_kernel_spmd(nc, maps, list(range(NCORES)), trace=trace)


def kernel(**inputs) -> np.ndarray:
    res = _run(inputs, trace=False)
    out = np.stack([res.results[i]["o_out"] for i in range(NCORES)])
    return out.reshape(B, OC, H, W).astype(np.float32)


# revision 13
# speedup vs baseline: 7.0192x; 1.2656x over previous
"""Trainium2 Bass kernel for nn_CSDKM_66417374265458 (dense_cnn).

Data-parallel over batch B=8 across 8 NeuronCores (one image per core, all
parameters replicated). The only cross-core communication is a 2KB AllReduce
of the BatchNorm batch statistics (sum / sum-of-squares per channel).

Per-core pipeline (all shapes per batch element):
  c4 (256,64,64), c5 (512,32,32)
  c4_proc = conv3x3(c4)                  -> shifted-window matmuls on PE
  c5_proc = conv1x1(c5) at 32x32, then nearest-upsample (1x1 conv commutes
            with nearest upsampling)
  fused   = c4_proc + up(c5_proc)        -> fused into the PSUM->SBUF move
  y       = conv1x1(fused); BN batch stats -> AllReduce -> X = silu(s*y+b)
  sim/gate path: adaptive pools as rectangle reductions, w_sim4^T w_sim5
            folded on host, tiny matmuls + softmax -> per-region 3x3 kernels
  dynfilter: out = sum_k kern[region,k] * shift_k(X) -> scaled-identity
            matmuls on PE accumulating in PSUM (region-rect free APs)
  out     = dynfilter(X) + conv1x1(conv1x1(fused))
"""
import sys

sys.path.insert(0, "/opt/trn_rl_repo")

import numpy as np

import concourse.bass as bass  # noqa: F401  (engine types referenced via nc)
import concourse.bacc as bacc
import concourse.tile as tile
from concourse import mybir
from concourse.bass_utils import run_bass_kernel_spmd

F32 = mybir.dt.float32
F32R = mybir.dt.float32r
ALU = mybir.AluOpType
ACTF = mybir.ActivationFunctionType
AX = mybir.AxisListType

B, C4, C5, H, W = 8, 256, 512, 64, 64
OC, FR, HID = 256, 128, 16
S, K2 = 3, 9
EPS = 1e-5
NCORES = 8
NPIX = H * W  # 4096
NSTAT = float(B * NPIX)  # BN sample count per channel

# Output-space region bands (start, len) for rows and cols: pidx regions.
BANDS = [(0, 22), (22, 21), (43, 21)]
# pool4 bins on the 64x64 grid (overlapping 22-wide intervals).
P4B = [(0, 22), (21, 22), (42, 22)]
# pool5 on the 32x32 grid: the upsampled 22-wide bin maps to interval sums
# over c5 rows; bin i = sum over listed (start, count) intervals, and a
# host-folded factor (uniform bins count each row twice).
P5IV = {0: [(0, 11)], 1: [(10, 12), (11, 10)], 2: [(21, 11)]}
P5FAC = {0: 2.0, 1: 1.0, 2: 2.0}

_CACHE = {}


def _build():
    nc = bacc.Bacc("TRN2", target_bir_lowering=False, debug=False,
                   num_devices=NCORES)

    # ---- DRAM I/O -------------------------------------------------------
    c4d = nc.dram_tensor("c4", [C4, 66 * 66], F32, kind="ExternalInput").ap()
    c5d = nc.dram_tensor("c5", [C5, 1024], F32, kind="ExternalInput").ap()
    wc4d = nc.dram_tensor("wc4t", [C4, 9, OC], F32, kind="ExternalInput").ap()
    wc1d = nc.dram_tensor("wc1t", [C5, OC], F32, kind="ExternalInput").ap()
    wtfd = nc.dram_tensor("wtft", [C4, OC], F32, kind="ExternalInput").ap()
    wrsd = nc.dram_tensor("wrst", [C4, FR], F32, kind="ExternalInput").ap()
    wprd = nc.dram_tensor("wprt", [FR, OC], F32, kind="ExternalInput").ap()
    mtd = nc.dram_tensor("mt", [C5, C4], F32, kind="ExternalInput").ap()
    w1d = nc.dram_tensor("w1", [HID], F32, kind="ExternalInput").ap()
    b1d = nc.dram_tensor("b1", [HID], F32, kind="ExternalInput").ap()
    w2d = nc.dram_tensor("w2t", [HID, K2], F32, kind="ExternalInput").ap()
    b2d = nc.dram_tensor("b2t", [K2, K2], F32, kind="ExternalInput").ap()
    sgd = nc.dram_tensor("sgp", [K2], F32, kind="ExternalInput").ap()
    gmd = nc.dram_tensor("gam", [OC], F32, kind="ExternalInput").ap()
    btd = nc.dram_tensor("bet", [OC], F32, kind="ExternalInput").ap()
    eyd = nc.dram_tensor("i128", [128, 128], F32, kind="ExternalInput").ap()
    outd = nc.dram_tensor("o_out", [OC, NPIX], F32, kind="ExternalOutput").ap()

    with tile.TileContext(nc) as tc:
        with (
            tc.tile_pool(name="big", bufs=3) as big,
            tc.tile_pool(name="ypool", bufs=2) as ypool,
            tc.tile_pool(name="pad", bufs=2) as pad,
            tc.tile_pool(name="c5pool", bufs=6) as c5pool,
            tc.tile_pool(name="wts", bufs=1) as wts,
            tc.tile_pool(name="small", bufs=1) as small,
            tc.tile_pool(name="scr", bufs=2) as scr,
            tc.tile_pool(name="idp", bufs=6) as idp,
            tc.tile_pool(name="ps8", bufs=8, space="PSUM") as ps8,
            tc.tile_pool(name="dram", bufs=1, space="DRAM") as dram,
        ):
            dma = nc.sync.dma_start

            # ---- weights / consts in (c5-conv path first) ------------
            wc1_sb = wts.tile([128, 4, OC], F32, tag="wc1")
            dma(wc1_sb[:].bitcast(F32R), wc1d.rearrange("(b p) o -> p b o", p=128).bitcast(F32R))
            c5_sb = []
            for icb in range(4):
                t = c5pool.tile([128, 1024], F32, tag="c5in", bufs=4,
                                name=f"c5in{icb}")
                dma(t[:].bitcast(F32R), c5d[icb * 128:(icb + 1) * 128].bitcast(F32R))
                c5_sb.append(t)
            c4p = []
            SPLIT = 40 * 66
            for cb in range(2):
                t = pad.tile([128, 66, 66], F32, tag="pad66", name=f"c4p{cb}")
                dma(t[:].rearrange("p a b -> p (a b)")[:, :SPLIT].bitcast(F32R),
                    c4d[cb * 128:(cb + 1) * 128, :SPLIT].bitcast(F32R))
                c4p.append(t)
            wc4_sb = []
            for icb in range(2):
                t = wts.tile([128, 9, OC], F32, tag=f"wc4_{icb}")
                dma(t[:].rearrange("p a b -> p (a b)").bitcast(F32R),
                    wc4d[icb * 128:(icb + 1) * 128].rearrange("p a b -> p (a b)").bitcast(F32R))
                wc4_sb.append(t)
            for cb in range(2):
                dma(c4p[cb][:].rearrange("p a b -> p (a b)")[:, SPLIT:].bitcast(F32R),
                    c4d[cb * 128:(cb + 1) * 128, SPLIT:].bitcast(F32R))
            wtf_sb = wts.tile([128, 2, OC], F32, tag="wtf")
            dma(wtf_sb[:].bitcast(F32R), wtfd.rearrange("(b p) o -> p b o", p=128).bitcast(F32R))
            wrs_sb = wts.tile([128, 2, FR], F32, tag="wrs")
            dma(wrs_sb[:].bitcast(F32R), wrsd.rearrange("(b p) o -> p b o", p=128).bitcast(F32R))
            wpr_sb = wts.tile([128, OC], F32, tag="wpr")
            dma(wpr_sb[:].bitcast(F32R), wprd.bitcast(F32R))
            mt_sb = wts.tile([128, 4, C4], F32, tag="mt")
            dma(mt_sb[:], mtd.rearrange("(b p) o -> p b o", p=128))
            eye_sb = wts.tile([128, 128], F32, tag="eye")
            dma(eye_sb[:], eyd)
            w1_sb = wts.tile([1, HID], F32, tag="w1")
            dma(w1_sb[:], w1d[None, :])
            b1_sb = wts.tile([HID, 1], F32, tag="b1")
            dma(b1_sb[:], b1d[:, None])
            w2_sb = wts.tile([HID, K2], F32, tag="w2")
            dma(w2_sb[:], w2d)
            b2_sb = wts.tile([K2, K2], F32, tag="b2")
            dma(b2_sb[:], b2d)
            sg_sb = wts.tile([1, K2], F32, tag="sg")
            dma(sg_sb[:], sgd[None, :])
            gam_sb, bet_sb = [], []
            for cb in range(2):
                g = wts.tile([128, 1], F32, tag=f"gam{cb}")
                dma(g[:], gmd[cb * 128:(cb + 1) * 128][:, None])
                gam_sb.append(g)
                bt = wts.tile([128, 1], F32, tag=f"bet{cb}")
                dma(bt[:], btd[cb * 128:(cb + 1) * 128][:, None])
                bet_sb.append(bt)
            ones_sb = wts.tile([128, 1], F32, tag="ones")
            nc.vector.memset(ones_sb[:], 1.0)

            zrow = wts.tile([128, 66], F32, tag="zrow")
            nc.vector.memset(zrow[:], 0.0)

            # ---- pool4: 9 overlapping 22x22 rect sums per ch block ---
            praw4 = []
            for cb in range(2):
                p4 = small.tile([128, K2], F32, tag=f"praw4_{cb}")
                for i, (r0, nr) in enumerate(P4B):
                    for j, (c0, ncc) in enumerate(P4B):
                        nc.vector.tensor_reduce(
                            p4[:, i * 3 + j: i * 3 + j + 1],
                            c4p[cb][:, r0 + 1:r0 + 1 + nr, c0 + 1:c0 + 1 + ncc],
                            AX.XY, ALU.add)
                praw4.append(p4)

            # ---- pool5: separable interval sums on the 32x32 grid ----
            praw5 = []
            for icb in range(4):
                v = c5_sb[icb][:].rearrange("p (h w) -> p h w", h=32)
                cs = small.tile([128, 3, 32], F32, tag=f"cs_{icb}")
                for j in range(3):
                    ivs = P5IV[j]
                    nc.vector.tensor_reduce(
                        cs[:, j, :][:, :, None], v[:, :, ivs[0][0]:ivs[0][0] + ivs[0][1]],
                        AX.X, ALU.add)
                    if len(ivs) > 1:
                        tmp = small.tile([128, 32], F32, tag=f"cstmp_{icb}")
                        nc.vector.tensor_reduce(
                            tmp[:, :, None], v[:, :, ivs[1][0]:ivs[1][0] + ivs[1][1]],
                            AX.X, ALU.add)
                        nc.vector.tensor_add(cs[:, j, :], cs[:, j, :], tmp[:])
                p5 = small.tile([128, K2], F32, tag=f"praw5_{icb}")
                for i in range(3):
                    ivs = P5IV[i]
                    for j in range(3):
                        sl = p5[:, i * 3 + j: i * 3 + j + 1]
                        nc.vector.tensor_reduce(
                            sl, cs[:, j, ivs[0][0]:ivs[0][0] + ivs[0][1]],
                            AX.X, ALU.add)
                        if len(ivs) > 1:
                            t1 = small.tile([128, 1], F32, tag=f"p5tmp_{icb}")
                            nc.vector.tensor_reduce(
                                t1[:], cs[:, j, ivs[1][0]:ivs[1][0] + ivs[1][1]],
                                AX.X, ALU.add)
                            nc.vector.tensor_add(sl, sl, t1[:])
                praw5.append(p5)

            # ---- c5 1x1 conv at 32x32 -------------------------------
            c5p_sb = []
            for cb in range(2):
                t = c5pool.tile([128, 1024], F32, tag="c5p", bufs=2)
                for pt in range(2):
                    ps = ps8.tile([128, 512], F32, tag="ps")
                    for icb in range(4):
                        nc.tensor.matmul(
                            ps[:],
                            wc1_sb[:, icb, cb * 128:(cb + 1) * 128].bitcast(F32R),
                            c5_sb[icb][:, pt * 512:(pt + 1) * 512].bitcast(F32R),
                            start=(icb == 0), stop=(icb == 3))
                    nc.scalar.copy(t[:, pt * 512:(pt + 1) * 512], ps[:])
                c5p_sb.append(t)

            # ---- materialize nearest-upsampled c5_proc ---------------
            # c5up shares slots with the later y tiles (disjoint lifetimes).
            c5up = []
            for cb in range(2):
                u = ypool.tile([128, NPIX], F32, tag="y", name=f"c5up{cb}")
                uv = u[:].rearrange("p (h a w b) -> p h a w b", h=32, a=2, b=2)
                sv = c5p_sb[cb][:].rearrange("p (h w) -> p h w", h=32)
                for a in range(2):
                    for bb in range(2):
                        nc.vector.tensor_copy(uv[:, :, a, :, bb], sv)
                c5up.append(u)

            # ---- conv3x3 + fused assembly (+ upsampled c5_proc) ------
            fused = []
            for cb in range(2):
                f = big.tile([128, NPIX], F32, tag="mid16")
                for pt in range(8):
                    ps = ps8.tile([128, 512], F32, tag="ps")
                    for icb in range(2):
                        for tap in range(9):
                            dy, dx = tap // 3, tap % 3
                            nc.tensor.matmul(
                                ps[:],
                                wc4_sb[icb][:, tap, cb * 128:(cb + 1) * 128].bitcast(F32R),
                                c4p[icb][:, pt * 8 + dy:pt * 8 + dy + 8, dx:dx + 64].bitcast(F32R),
                                start=(icb == 0 and tap == 0),
                                stop=(icb == 1 and tap == 8))
                    nc.vector.scalar_tensor_tensor(
                        out=f[:, pt * 512:(pt + 1) * 512].bitcast(F32R),
                        in0=ps[:],
                        scalar=1.0, in1=c5up[cb][:, pt * 512:(pt + 1) * 512],
                        op0=ALU.mult, op1=ALU.add)
                fused.append(f)

            # ---- to_fuse 1x1 conv -> y (+ stats partials) ------------
            y_sb, ysum_p, ysq_p = [], [], []
            for cb in range(2):
                y = ypool.tile([128, NPIX], F32, tag="y")
                su = small.tile([128, 8], F32, tag=f"ysum_{cb}")
                sq = small.tile([128, 8], F32, tag=f"ysq_{cb}")
                for pt in range(8):
                    ps = ps8.tile([128, 512], F32, tag="ps")
                    for icb in range(2):
                        nc.tensor.matmul(
                            ps[:],
                            wtf_sb[:, icb, cb * 128:(cb + 1) * 128].bitcast(F32R),
                            fused[icb][:, pt * 512:(pt + 1) * 512].bitcast(F32R),
                            start=(icb == 0), stop=(icb == 1))
                    nc.scalar.activation(
                        y[:, pt * 512:(pt + 1) * 512], ps[:], ACTF.Copy,
                        accum_out=su[:, pt:pt + 1])
                    sc = scr.tile([128, 512], F32, tag="sqscr")
                    nc.scalar.activation(sc[:], ps[:], ACTF.Square,
                                         accum_out=sq[:, pt:pt + 1])
                y_sb.append(y)
                ysum_p.append(su)
                ysq_p.append(sq)

            # ---- sim / gating / per-region kernels -------------------
            mp5_ps = []
            for cb in range(2):
                mp = ps8.tile([128, K2], F32, tag="ps")
                for icb in range(4):
                    nc.tensor.matmul(
                        mp[:], mt_sb[:, icb, cb * 128:(cb + 1) * 128],
                        praw5[icb][:], start=(icb == 0), stop=(icb == 3))
                mp5_ps.append(mp)
            sim_ps = ps8.tile([1, K2], F32, tag="ps")
            for cb in range(2):
                e = small.tile([128, K2], F32, tag=f"esim_{cb}")
                nc.vector.tensor_mul(e[:], praw4[cb][:], mp5_ps[cb][:])
                nc.tensor.matmul(sim_ps[:], ones_sb[:], e[:],
                                 start=(cb == 0), stop=(cb == 1))
            gated = small.tile([1, K2], F32, tag="gated")
            nc.vector.tensor_mul(gated[:], sim_ps[:], sg_sb[:])
            h_ps = ps8.tile([HID, K2], F32, tag="ps")
            nc.tensor.matmul(h_ps[:], w1_sb[:], gated[:])
            h_sb = small.tile([HID, K2], F32, tag="h")
            nc.scalar.activation(h_sb[:], h_ps[:], ACTF.Relu, bias=b1_sb[:])
            lg_ps = ps8.tile([K2, K2], F32, tag="ps")
            nc.tensor.matmul(lg_ps[:], h_sb[:], w2_sb[:])
            lg2 = small.tile([K2, K2], F32, tag="lg2")
            nc.vector.tensor_add(lg2[:], lg_ps[:], b2_sb[:])
            nmx = small.tile([K2, 1], F32, tag="nmx")
            nc.vector.tensor_reduce(nmx[:], lg2[:], AX.X, ALU.max, negate=True)
            esb = small.tile([K2, K2], F32, tag="esb")
            esum = small.tile([K2, 1], F32, tag="esum")
            nc.scalar.activation(esb[:], lg2[:], ACTF.Exp, bias=nmx[:],
                                 accum_out=esum[:])
            rs = small.tile([K2, 1], F32, tag="rs")
            nc.vector.reciprocal(rs[:], esum[:])
            kern = small.tile([K2, K2], F32, tag="kern")
            nc.vector.tensor_scalar_mul(kern[:], esb[:], rs[:])
            kd = dram.tile([K2, K2], F32, tag="kdram")
            dma(kd[:], kern[:])
            kbc = wts.tile([128, 81], F32, tag="kbc")
            dma(kbc[:], kd[:].rearrange("a b -> (a b)")[None, :].broadcast_to([128, 81]))

            # ---- BN stats -> AllReduce -> scale/bias -> X = silu -----
            stats = small.tile([128, 4], F32, tag="stats")
            for cb in range(2):
                nc.vector.tensor_reduce(stats[:, cb:cb + 1], ysum_p[cb][:],
                                        AX.X, ALU.add)
                nc.vector.tensor_reduce(stats[:, 2 + cb:3 + cb], ysq_p[cb][:],
                                        AX.X, ALU.add)
            stin = dram.tile([128, 4], F32, tag="stin")
            stout = dram.tile([128, 4], F32, tag="stout")
            dma(stin[:], stats[:])
            nc.gpsimd.collective_compute(
                "AllReduce", ALU.add,
                replica_groups=[list(range(NCORES))],
                ins=[stin.opt()], outs=[stout.opt()])
            stats2 = small.tile([128, 4], F32, tag="stats2")
            dma(stats2[:], stout[:])

            xp = []
            s_t, b_t = [], []
            for cb in range(2):
                mu = small.tile([128, 1], F32, tag=f"mu{cb}")
                nc.vector.tensor_scalar_mul(mu[:], stats2[:, cb:cb + 1], 1.0 / NSTAT)
                musq = small.tile([128, 1], F32, tag=f"musq{cb}")
                nc.vector.tensor_mul(musq[:], mu[:], mu[:])
                # musq - EPS, so that sq/N - (musq - EPS) = var + EPS
                nc.vector.tensor_scalar_add(musq[:], musq[:], -EPS)
                var = small.tile([128, 1], F32, tag=f"var{cb}")
                nc.vector.scalar_tensor_tensor(
                    out=var[:], in0=stats2[:, 2 + cb:3 + cb], scalar=1.0 / NSTAT,
                    in1=musq[:], op0=ALU.mult, op1=ALU.subtract)
                sd = small.tile([128, 1], F32, tag=f"sd{cb}")
                nc.scalar.activation(sd[:], var[:], ACTF.Sqrt)
                rinv = small.tile([128, 1], F32, tag=f"rinv{cb}")
                nc.vector.reciprocal(rinv[:], sd[:])
                st = small.tile([128, 1], F32, tag=f"sbn{cb}")
                nc.vector.tensor_mul(st[:], gam_sb[cb][:], rinv[:])
                t1 = small.tile([128, 1], F32, tag=f"t1{cb}")
                nc.vector.tensor_scalar_mul(t1[:], mu[:], st[:])
                bt = small.tile([128, 1], F32, tag=f"bbn{cb}")
                nc.vector.tensor_sub(bt[:], bet_sb[cb][:], t1[:])
                s_t.append(st)
                b_t.append(bt)
            for cb in range(2):
                x = pad.tile([128, 66, 66], F32, tag="pad66")
                nc.vector.tensor_copy(x[:, 0, :].bitcast(F32R), zrow[:])
                nc.vector.tensor_copy(x[:, 65, :].bitcast(F32R), zrow[:])
                xs = x[:].rearrange("p a b -> p (a b)")[:, 65:65 + 65 * 66]
                nc.vector.tensor_copy(
                    xs.rearrange("p (r t) -> p r t", t=66)[:, :, 0:2].bitcast(F32R),
                    zrow[:, None, 0:2].broadcast_to([128, 65, 2]))
                yv = y_sb[cb][:].rearrange("p (h w) -> p h w", h=H)
                for (ra, rb) in ((0, 24), (24, 44), (44, 64)):
                    nc.scalar.activation(
                        x[:, 1 + ra:1 + rb, 1:65].bitcast(F32R),
                        yv[:, ra:rb, :],
                        ACTF.Silu, bias=b_t[cb][:], scale=s_t[cb][:])
                xp.append(x)

            # ---- fused_red = proj(reshape(fused)) --------------------
            t_sb = big.tile([128, NPIX], F32, tag="mid16")
            for pt in range(8):
                ps = ps8.tile([128, 512], F32, tag="ps")
                for icb in range(2):
                    nc.tensor.matmul(
                        ps[:], wrs_sb[:, icb, :].bitcast(F32R),
                        fused[icb][:, pt * 512:(pt + 1) * 512].bitcast(F32R),
                        start=(icb == 0), stop=(icb == 1))
                nc.scalar.copy(t_sb[:, pt * 512:(pt + 1) * 512].bitcast(F32R), ps[:])
            fr = []
            for cb in range(2):
                f = big.tile([128, NPIX], F32, tag="mid16")
                for pt in range(8):
                    ps = ps8.tile([128, 512], F32, tag="ps")
                    nc.tensor.matmul(
                        ps[:], wpr_sb[:, cb * 128:(cb + 1) * 128].bitcast(F32R),
                        t_sb[:, pt * 512:(pt + 1) * 512].bitcast(F32R))
                    nc.scalar.copy(f[:, pt * 512:(pt + 1) * 512], ps[:])
                fr.append(f)

            # ---- dynamic filter + final add --------------------------
            # fp32r matmuls need even free sizes: every region computed as a
            # uniform 22x22 window (overlapping a row/col into the neighbor
            # band with this region's weights); the final add consumes only
            # the correct sub-rectangle. Output DMA'd per row-band so the
            # store overlaps later bands' compute.
            DBANDS = [(0, 22, 0, 0), (22, 21, 22, 0), (43, 21, 42, 1)]
            for ry, (r0, nr, gr, orow) in enumerate(DBANDS):
                for rx, (c0, ncc, gc, ocol) in enumerate(DBANDS):
                    reg = ry * 3 + rx
                    pds = [ps8.tile([128, 484], F32, tag="ps",
                                    name=f"pd{reg}_{i}") for i in range(2)]
                    for tap in range(9):
                        dy, dx = tap // 3, tap % 3
                        rk = reg * 9 + tap
                        idt = idp.tile([128, 128], F32, tag="idt",
                                       name=f"idt{rk}")
                        if tap % 2 == 0:
                            nc.vector.tensor_scalar_mul(
                                idt[:].bitcast(F32R), eye_sb[:],
                                kbc[:, rk:rk + 1])
                        else:
                            nc.scalar.mul(idt[:].bitcast(F32R), eye_sb[:],
                                          kbc[:, rk:rk + 1])
                        for cb in range(2):
                            nc.tensor.matmul(
                                pds[cb][:], idt[:].bitcast(F32R),
                                xp[cb][:, gr + dy:gr + dy + 22,
                                       gc + dx:gc + dx + 22].bitcast(F32R),
                                start=(tap == 0), stop=(tap == 8))
                    for cb in range(2):
                        fv = fr[cb][:].rearrange("p (h w) -> p h w", h=H)
                        pv = pds[cb][:].rearrange("p (a b) -> p a b", a=22)
                        nc.vector.tensor_add(
                            fv[:, r0:r0 + nr, c0:c0 + ncc],
                            pv[:, orow:orow + nr, ocol:ocol + ncc],
                            fv[:, r0:r0 + nr, c0:c0 + ncc])
                for cb in range(2):
                    dma(outd[cb * 128:(cb + 1) * 128, r0 * 64:(r0 + nr) * 64],
                        fr[cb][:, r0 * 64:(r0 + nr) * 64])

    nc.compile()
    return nc


def _prep_inputs(inputs):
    """Host-side parameter folding + per-core input maps."""
    f = np.float32
    c4r = np.asarray(inputs["c4"], f).reshape(B, C4, H, W)
    c4 = np.zeros((B, C4, 66, 66), f)
    c4[:, :, 1:65, 1:65] = c4r
    c4 = c4.reshape(B, C4, 66 * 66)
    c5 = np.ascontiguousarray(inputs["c5"], f).reshape(B, C5, 1024)
    wc4 = np.ascontiguousarray(
        np.transpose(np.asarray(inputs["w_c4_proc"], f).reshape(OC, C4, 9),
                     (1, 2, 0)))                      # (ic, tap, oc)
    wc1 = np.ascontiguousarray(np.asarray(inputs["w_conv1"], f).reshape(OC, C5).T)
    wtf = np.ascontiguousarray(np.asarray(inputs["w_to_fuse"], f).reshape(OC, C4).T)
    wrs = np.ascontiguousarray(np.asarray(inputs["w_reshape"], f).reshape(FR, C4).T)
    wpr = np.ascontiguousarray(np.asarray(inputs["w_proj"], f).reshape(OC, FR).T)
    w4 = np.asarray(inputs["w_sim4"], f).reshape(64, C4)
    w5 = np.asarray(inputs["w_sim5"], f).reshape(64, C5)
    mt = np.ascontiguousarray(w5.T @ w4)              # (c5, c4) = (W4^T W5)^T
    sig = 1.0 / (1.0 + np.exp(-np.asarray(inputs["mask_raw"], np.float64)))
    fac = np.array([P5FAC[i] * P5FAC[j] for i in range(3) for j in range(3)],
                   np.float64)
    sgp = (sig * fac / (484.0 * 484.0)).astype(f)
    maps = []
    shared = dict(
        wc4t=wc4, wc1t=wc1, wtft=wtf, wrst=wrs, wprt=wpr, mt=mt,
        w1=np.ascontiguousarray(np.asarray(inputs["kg_w1"], f).reshape(HID)),
        b1=np.ascontiguousarray(np.asarray(inputs["kg_b1"], f)),
        w2t=np.ascontiguousarray(np.asarray(inputs["kg_w2"], f).T),
        b2t=np.ascontiguousarray(np.tile(np.asarray(inputs["kg_b2"], f), (K2, 1))),
        sgp=sgp,
        gam=np.ascontiguousarray(np.asarray(inputs["bn_gamma"], f)),
        bet=np.ascontiguousarray(np.asarray(inputs["bn_beta"], f)),
        i128=np.eye(128, dtype=f),
    )
    for b in range(B):
        m = dict(shared)
        m["c4"] = np.ascontiguousarray(c4[b])
        m["c5"] = np.ascontiguousarray(c5[b])
        maps.append(m)
    return maps


def _run(inputs, trace=False):
    if "nc" not in _CACHE:
        _CACHE["nc"] = _build()
    nc = _CACHE["nc"]
    maps = _prep_inputs(inputs)
    return run_bass_kernel_spmd(nc, maps, list(range(NCORES)), trace=trace)


def kernel(**inputs) -> np.ndarray:
    res = _run(inputs, trace=False)
    out = np.stack([res.results[i]["o_out"] for i in range(NCORES)])
    return out.reshape(B, OC, H, W).astype(np.float32)


# revision 14
# speedup vs baseline: 7.1638x; 1.0206x over previous
"""Trainium2 Bass kernel for nn_CSDKM_66417374265458 (dense_cnn).

Data-parallel over batch B=8 across 8 NeuronCores (one image per core, all
parameters replicated). The only cross-core communication is a 2KB AllReduce
of the BatchNorm batch statistics (sum / sum-of-squares per channel).

Per-core pipeline (all shapes per batch element):
  c4 (256,64,64), c5 (512,32,32)
  c4_proc = conv3x3(c4)                  -> shifted-window matmuls on PE
  c5_proc = conv1x1(c5) at 32x32, then nearest-upsample (1x1 conv commutes
            with nearest upsampling)
  fused   = c4_proc + up(c5_proc)        -> fused into the PSUM->SBUF move
  y       = conv1x1(fused); BN batch stats -> AllReduce -> X = silu(s*y+b)
  sim/gate path: adaptive pools as rectangle reductions, w_sim4^T w_sim5
            folded on host, tiny matmuls + softmax -> per-region 3x3 kernels
  dynfilter: out = sum_k kern[region,k] * shift_k(X) -> scaled-identity
            matmuls on PE accumulating in PSUM (region-rect free APs)
  out     = dynfilter(X) + conv1x1(conv1x1(fused))
"""
import sys

sys.path.insert(0, "/opt/trn_rl_repo")

import numpy as np

import concourse.bass as bass  # noqa: F401  (engine types referenced via nc)
import concourse.bacc as bacc
import concourse.tile as tile
from concourse import mybir
from concourse.bass_utils import run_bass_kernel_spmd

F32 = mybir.dt.float32
F32R = mybir.dt.float32r
ALU = mybir.AluOpType
ACTF = mybir.ActivationFunctionType
AX = mybir.AxisListType

B, C4, C5, H, W = 8, 256, 512, 64, 64
OC, FR, HID = 256, 128, 16
S, K2 = 3, 9
EPS = 1e-5
NCORES = 8
NPIX = H * W  # 4096
NSTAT = float(B * NPIX)  # BN sample count per channel

# Output-space region bands (start, len) for rows and cols: pidx regions.
BANDS = [(0, 22), (22, 21), (43, 21)]
# pool4 bins on the 64x64 grid (overlapping 22-wide intervals).
P4B = [(0, 22), (21, 22), (42, 22)]
# pool5 on the 32x32 grid: the upsampled 22-wide bin maps to interval sums
# over c5 rows; bin i = sum over listed (start, count) intervals, and a
# host-folded factor (uniform bins count each row twice).
P5IV = {0: [(0, 11)], 1: [(10, 12), (11, 10)], 2: [(21, 11)]}
P5FAC = {0: 2.0, 1: 1.0, 2: 2.0}

_CACHE = {}


def _build():
    nc = bacc.Bacc("TRN2", target_bir_lowering=False, debug=False,
                   num_devices=NCORES)

    # ---- DRAM I/O -------------------------------------------------------
    c4d = nc.dram_tensor("c4", [C4, 66 * 66], F32, kind="ExternalInput").ap()
    c5d = nc.dram_tensor("c5", [C5, 1024], F32, kind="ExternalInput").ap()
    wc4d = nc.dram_tensor("wc4t", [C4, 9, OC], F32, kind="ExternalInput").ap()
    wc1d = nc.dram_tensor("wc1t", [C5, OC], F32, kind="ExternalInput").ap()
    wtfd = nc.dram_tensor("wtft", [C4, OC], F32, kind="ExternalInput").ap()
    wrsd = nc.dram_tensor("wrst", [C4, FR], F32, kind="ExternalInput").ap()
    wprd = nc.dram_tensor("wprt", [FR, OC], F32, kind="ExternalInput").ap()
    mtd = nc.dram_tensor("mt", [C5, C4], F32, kind="ExternalInput").ap()
    w1d = nc.dram_tensor("w1", [HID], F32, kind="ExternalInput").ap()
    b1d = nc.dram_tensor("b1", [HID], F32, kind="ExternalInput").ap()
    w2d = nc.dram_tensor("w2t", [HID, K2], F32, kind="ExternalInput").ap()
    b2d = nc.dram_tensor("b2t", [K2, K2], F32, kind="ExternalInput").ap()
    sgd = nc.dram_tensor("sgp", [K2], F32, kind="ExternalInput").ap()
    gmd = nc.dram_tensor("gam", [OC], F32, kind="ExternalInput").ap()
    btd = nc.dram_tensor("bet", [OC], F32, kind="ExternalInput").ap()
    eyd = nc.dram_tensor("i128", [128, 128], F32, kind="ExternalInput").ap()
    outd = nc.dram_tensor("o_out", [OC, NPIX], F32, kind="ExternalOutput").ap()

    with tile.TileContext(nc) as tc:
        with (
            tc.tile_pool(name="big", bufs=3) as big,
            tc.tile_pool(name="ypool", bufs=2) as ypool,
            tc.tile_pool(name="pad", bufs=2) as pad,
            tc.tile_pool(name="c5pool", bufs=6) as c5pool,
            tc.tile_pool(name="wts", bufs=1) as wts,
            tc.tile_pool(name="small", bufs=1) as small,
            tc.tile_pool(name="scr", bufs=2) as scr,
            tc.tile_pool(name="idp", bufs=10) as idp,
            tc.tile_pool(name="ps8", bufs=8, space="PSUM") as ps8,
            tc.tile_pool(name="dram", bufs=1, space="DRAM") as dram,
        ):
            dma = nc.sync.dma_start

            # ---- weights / consts in (c5-conv path first) ------------
            wc1_sb = wts.tile([128, 4, OC], F32, tag="wc1")
            dma(wc1_sb[:].bitcast(F32R), wc1d.rearrange("(b p) o -> p b o", p=128).bitcast(F32R))
            c5_sb = []
            for icb in range(4):
                t = c5pool.tile([128, 1024], F32, tag="c5in", bufs=4,
                                name=f"c5in{icb}")
                dma(t[:].bitcast(F32R), c5d[icb * 128:(icb + 1) * 128].bitcast(F32R))
                c5_sb.append(t)
            c4p = []
            S1, S2 = 14 * 66, 40 * 66
            for cb in range(2):
                t = pad.tile([128, 66, 66], F32, tag="pad66", name=f"c4p{cb}")
                dma(t[:].rearrange("p a b -> p (a b)")[:, :S1].bitcast(F32R),
                    c4d[cb * 128:(cb + 1) * 128, :S1].bitcast(F32R))
                c4p.append(t)
            wc4_sb = []
            for icb in range(2):
                t = wts.tile([128, 9, OC], F32, tag=f"wc4_{icb}")
                dma(t[:].rearrange("p a b -> p (a b)").bitcast(F32R),
                    wc4d[icb * 128:(icb + 1) * 128].rearrange("p a b -> p (a b)").bitcast(F32R))
                wc4_sb.append(t)
            for cb in range(2):
                dma(c4p[cb][:].rearrange("p a b -> p (a b)")[:, S1:S2].bitcast(F32R),
                    c4d[cb * 128:(cb + 1) * 128, S1:S2].bitcast(F32R))
            for cb in range(2):
                dma(c4p[cb][:].rearrange("p a b -> p (a b)")[:, S2:].bitcast(F32R),
                    c4d[cb * 128:(cb + 1) * 128, S2:].bitcast(F32R))
            wtf_sb = wts.tile([128, 2, OC], F32, tag="wtf")
            dma(wtf_sb[:].bitcast(F32R), wtfd.rearrange("(b p) o -> p b o", p=128).bitcast(F32R))
            wrs_sb = wts.tile([128, 2, FR], F32, tag="wrs")
            dma(wrs_sb[:].bitcast(F32R), wrsd.rearrange("(b p) o -> p b o", p=128).bitcast(F32R))
            wpr_sb = wts.tile([128, OC], F32, tag="wpr")
            dma(wpr_sb[:].bitcast(F32R), wprd.bitcast(F32R))
            mt_sb = wts.tile([128, 4, C4], F32, tag="mt")
            dma(mt_sb[:], mtd.rearrange("(b p) o -> p b o", p=128))
            eye_sb = wts.tile([128, 128], F32, tag="eye")
            dma(eye_sb[:], eyd)
            w1_sb = wts.tile([1, HID], F32, tag="w1")
            dma(w1_sb[:], w1d[None, :])
            b1_sb = wts.tile([HID, 1], F32, tag="b1")
            dma(b1_sb[:], b1d[:, None])
            w2_sb = wts.tile([HID, K2], F32, tag="w2")
            dma(w2_sb[:], w2d)
            b2_sb = wts.tile([K2, K2], F32, tag="b2")
            dma(b2_sb[:], b2d)
            sg_sb = wts.tile([1, K2], F32, tag="sg")
            dma(sg_sb[:], sgd[None, :])
            gam_sb, bet_sb = [], []
            for cb in range(2):
                g = wts.tile([128, 1], F32, tag=f"gam{cb}")
                dma(g[:], gmd[cb * 128:(cb + 1) * 128][:, None])
                gam_sb.append(g)
                bt = wts.tile([128, 1], F32, tag=f"bet{cb}")
                dma(bt[:], btd[cb * 128:(cb + 1) * 128][:, None])
                bet_sb.append(bt)
            ones_sb = wts.tile([128, 1], F32, tag="ones")
            nc.vector.memset(ones_sb[:], 1.0)

            zrow = wts.tile([128, 66], F32, tag="zrow")
            nc.vector.memset(zrow[:], 0.0)
            warm_in = dram.tile([128, 1], F32, tag="warm_in")
            warm_out = dram.tile([128, 1], F32, tag="warm_out")
            dma(warm_in[:], zrow[:, 0:1])
            nc.gpsimd.collective_compute(
                "AllReduce", ALU.add,
                replica_groups=[list(range(NCORES))],
                ins=[warm_in.opt()], outs=[warm_out.opt()])

            # ---- pool4: 9 overlapping 22x22 rect sums per ch block ---
            praw4 = []
            for cb in range(2):
                p4 = small.tile([128, K2], F32, tag=f"praw4_{cb}")
                for i, (r0, nr) in enumerate(P4B):
                    for j, (c0, ncc) in enumerate(P4B):
                        nc.vector.tensor_reduce(
                            p4[:, i * 3 + j: i * 3 + j + 1],
                            c4p[cb][:, r0 + 1:r0 + 1 + nr, c0 + 1:c0 + 1 + ncc],
                            AX.XY, ALU.add)
                praw4.append(p4)

            # ---- pool5: separable interval sums on the 32x32 grid ----
            praw5 = []
            for icb in range(4):
                v = c5_sb[icb][:].rearrange("p (h w) -> p h w", h=32)
                cs = small.tile([128, 3, 32], F32, tag=f"cs_{icb}")
                for j in range(3):
                    ivs = P5IV[j]
                    nc.vector.tensor_reduce(
                        cs[:, j, :][:, :, None], v[:, :, ivs[0][0]:ivs[0][0] + ivs[0][1]],
                        AX.X, ALU.add)
                    if len(ivs) > 1:
                        tmp = small.tile([128, 32], F32, tag=f"cstmp_{icb}")
                        nc.vector.tensor_reduce(
                            tmp[:, :, None], v[:, :, ivs[1][0]:ivs[1][0] + ivs[1][1]],
                            AX.X, ALU.add)
                        nc.vector.tensor_add(cs[:, j, :], cs[:, j, :], tmp[:])
                p5 = small.tile([128, K2], F32, tag=f"praw5_{icb}")
                for i in range(3):
                    ivs = P5IV[i]
                    for j in range(3):
                        sl = p5[:, i * 3 + j: i * 3 + j + 1]
                        nc.vector.tensor_reduce(
                            sl, cs[:, j, ivs[0][0]:ivs[0][0] + ivs[0][1]],
                            AX.X, ALU.add)
                        if len(ivs) > 1:
                            t1 = small.tile([128, 1], F32, tag=f"p5tmp_{icb}")
                            nc.vector.tensor_reduce(
                                t1[:], cs[:, j, ivs[1][0]:ivs[1][0] + ivs[1][1]],
                                AX.X, ALU.add)
                            nc.vector.tensor_add(sl, sl, t1[:])
                praw5.append(p5)

            # ---- c5 1x1 conv at 32x32 -------------------------------
            c5p_sb = []
            for cb in range(2):
                t = c5pool.tile([128, 1024], F32, tag="c5p", bufs=2)
                for pt in range(2):
                    ps = ps8.tile([128, 512], F32, tag="ps")
                    for icb in range(4):
                        nc.tensor.matmul(
                            ps[:],
                            wc1_sb[:, icb, cb * 128:(cb + 1) * 128].bitcast(F32R),
                            c5_sb[icb][:, pt * 512:(pt + 1) * 512].bitcast(F32R),
                            start=(icb == 0), stop=(icb == 3))
                    nc.scalar.copy(t[:, pt * 512:(pt + 1) * 512], ps[:])
                c5p_sb.append(t)

            # ---- materialize nearest-upsampled c5_proc ---------------
            # c5up shares slots with the later y tiles (disjoint lifetimes).
            c5up = []
            for cb in range(2):
                u = ypool.tile([128, NPIX], F32, tag="y", name=f"c5up{cb}")
                uv = u[:].rearrange("p (h a w b) -> p h a w b", h=32, a=2, b=2)
                sv = c5p_sb[cb][:].rearrange("p (h w) -> p h w", h=32)
                for a in range(2):
                    for bb in range(2):
                        nc.vector.tensor_copy(uv[:, :, a, :, bb], sv)
                c5up.append(u)

            # ---- conv3x3 + fused assembly (+ upsampled c5_proc) ------
            fused = []
            for cb in range(2):
                f = big.tile([128, NPIX], F32, tag="mid16")
                for pt in range(8):
                    ps = ps8.tile([128, 512], F32, tag="ps")
                    for icb in range(2):
                        for tap in range(9):
                            dy, dx = tap // 3, tap % 3
                            nc.tensor.matmul(
                                ps[:],
                                wc4_sb[icb][:, tap, cb * 128:(cb + 1) * 128].bitcast(F32R),
                                c4p[icb][:, pt * 8 + dy:pt * 8 + dy + 8, dx:dx + 64].bitcast(F32R),
                                start=(icb == 0 and tap == 0),
                                stop=(icb == 1 and tap == 8))
                    nc.vector.scalar_tensor_tensor(
                        out=f[:, pt * 512:(pt + 1) * 512].bitcast(F32R),
                        in0=ps[:],
                        scalar=1.0, in1=c5up[cb][:, pt * 512:(pt + 1) * 512],
                        op0=ALU.mult, op1=ALU.add)
                fused.append(f)

            # ---- to_fuse 1x1 conv -> y (+ stats partials) ------------
            y_sb, ysum_p, ysq_p = [], [], []
            for cb in range(2):
                y = ypool.tile([128, NPIX], F32, tag="y")
                su = small.tile([128, 8], F32, tag=f"ysum_{cb}")
                sq = small.tile([128, 8], F32, tag=f"ysq_{cb}")
                for pt in range(8):
                    ps = ps8.tile([128, 512], F32, tag="ps")
                    for icb in range(2):
                        nc.tensor.matmul(
                            ps[:],
                            wtf_sb[:, icb, cb * 128:(cb + 1) * 128].bitcast(F32R),
                            fused[icb][:, pt * 512:(pt + 1) * 512].bitcast(F32R),
                            start=(icb == 0), stop=(icb == 1))
                    nc.scalar.activation(
                        y[:, pt * 512:(pt + 1) * 512], ps[:], ACTF.Copy,
                        accum_out=su[:, pt:pt + 1])
                    sc = scr.tile([128, 512], F32, tag="sqscr")
                    nc.scalar.activation(sc[:], ps[:], ACTF.Square,
                                         accum_out=sq[:, pt:pt + 1])
                y_sb.append(y)
                ysum_p.append(su)
                ysq_p.append(sq)

            # ---- sim / gating / per-region kernels -------------------
            mp5_ps = []
            for cb in range(2):
                mp = ps8.tile([128, K2], F32, tag="ps")
                for icb in range(4):
                    nc.tensor.matmul(
                        mp[:], mt_sb[:, icb, cb * 128:(cb + 1) * 128],
                        praw5[icb][:], start=(icb == 0), stop=(icb == 3))
                mp5_ps.append(mp)
            sim_ps = ps8.tile([1, K2], F32, tag="ps")
            for cb in range(2):
                e = small.tile([128, K2], F32, tag=f"esim_{cb}")
                nc.vector.tensor_mul(e[:], praw4[cb][:], mp5_ps[cb][:])
                nc.tensor.matmul(sim_ps[:], ones_sb[:], e[:],
                                 start=(cb == 0), stop=(cb == 1))
            gated = small.tile([1, K2], F32, tag="gated")
            nc.vector.tensor_mul(gated[:], sim_ps[:], sg_sb[:])
            h_ps = ps8.tile([HID, K2], F32, tag="ps")
            nc.tensor.matmul(h_ps[:], w1_sb[:], gated[:])
            h_sb = small.tile([HID, K2], F32, tag="h")
            nc.scalar.activation(h_sb[:], h_ps[:], ACTF.Relu, bias=b1_sb[:])
            lg_ps = ps8.tile([K2, K2], F32, tag="ps")
            nc.tensor.matmul(lg_ps[:], h_sb[:], w2_sb[:])
            lg2 = small.tile([K2, K2], F32, tag="lg2")
            nc.vector.tensor_add(lg2[:], lg_ps[:], b2_sb[:])
            nmx = small.tile([K2, 1], F32, tag="nmx")
            nc.vector.tensor_reduce(nmx[:], lg2[:], AX.X, ALU.max, negate=True)
            esb = small.tile([K2, K2], F32, tag="esb")
            esum = small.tile([K2, 1], F32, tag="esum")
            nc.scalar.activation(esb[:], lg2[:], ACTF.Exp, bias=nmx[:],
                                 accum_out=esum[:])
            rs = small.tile([K2, 1], F32, tag="rs")
            nc.vector.reciprocal(rs[:], esum[:])
            kern = small.tile([K2, K2], F32, tag="kern")
            nc.vector.tensor_scalar_mul(kern[:], esb[:], rs[:])
            kd = dram.tile([K2, K2], F32, tag="kdram")
            dma(kd[:], kern[:])
            kbc = wts.tile([128, 81], F32, tag="kbc")
            dma(kbc[:], kd[:].rearrange("a b -> (a b)")[None, :].broadcast_to([128, 81]))

            # ---- BN stats -> AllReduce -> scale/bias -> X = silu -----
            stats = small.tile([128, 4], F32, tag="stats")
            for cb in range(2):
                nc.vector.tensor_reduce(stats[:, cb:cb + 1], ysum_p[cb][:],
                                        AX.X, ALU.add)
                nc.vector.tensor_reduce(stats[:, 2 + cb:3 + cb], ysq_p[cb][:],
                                        AX.X, ALU.add)
            stin = dram.tile([128, 4], F32, tag="stin")
            stout = dram.tile([128, 4], F32, tag="stout")
            dma(stin[:], stats[:])
            nc.gpsimd.collective_compute(
                "AllReduce", ALU.add,
                replica_groups=[list(range(NCORES))],
                ins=[stin.opt()], outs=[stout.opt()])
            stats2 = small.tile([128, 4], F32, tag="stats2")
            dma(stats2[:], stout[:])

            xp = []
            s_t, b_t = [], []
            for cb in range(2):
                mu = small.tile([128, 1], F32, tag=f"mu{cb}")
                nc.vector.tensor_scalar_mul(mu[:], stats2[:, cb:cb + 1], 1.0 / NSTAT)
                musq = small.tile([128, 1], F32, tag=f"musq{cb}")
                nc.vector.tensor_mul(musq[:], mu[:], mu[:])
                # musq - EPS, so that sq/N - (musq - EPS) = var + EPS
                nc.vector.tensor_scalar_add(musq[:], musq[:], -EPS)
                var = small.tile([128, 1], F32, tag=f"var{cb}")
                nc.vector.scalar_tensor_tensor(
                    out=var[:], in0=stats2[:, 2 + cb:3 + cb], scalar=1.0 / NSTAT,
                    in1=musq[:], op0=ALU.mult, op1=ALU.subtract)
                sd = small.tile([128, 1], F32, tag=f"sd{cb}")
                nc.scalar.activation(sd[:], var[:], ACTF.Sqrt)
                rinv = small.tile([128, 1], F32, tag=f"rinv{cb}")
                nc.vector.reciprocal(rinv[:], sd[:])
                st = small.tile([128, 1], F32, tag=f"sbn{cb}")
                nc.vector.tensor_mul(st[:], gam_sb[cb][:], rinv[:])
                t1 = small.tile([128, 1], F32, tag=f"t1{cb}")
                nc.vector.tensor_scalar_mul(t1[:], mu[:], st[:])
                bt = small.tile([128, 1], F32, tag=f"bbn{cb}")
                nc.vector.tensor_sub(bt[:], bet_sb[cb][:], t1[:])
                s_t.append(st)
                b_t.append(bt)
            for cb in range(2):
                x = pad.tile([128, 66, 66], F32, tag="pad66")
                nc.vector.tensor_copy(x[:, 0, :].bitcast(F32R), zrow[:])
                nc.vector.tensor_copy(x[:, 65, :].bitcast(F32R), zrow[:])
                xs = x[:].rearrange("p a b -> p (a b)")[:, 65:65 + 65 * 66]
                nc.vector.tensor_copy(
                    xs.rearrange("p (r t) -> p r t", t=66)[:, :, 0:2].bitcast(F32R),
                    zrow[:, None, 0:2].broadcast_to([128, 65, 2]))
                yv = y_sb[cb][:].rearrange("p (h w) -> p h w", h=H)
                for (ra, rb) in ((0, 24), (24, 44), (44, 64)):
                    nc.scalar.activation(
                        x[:, 1 + ra:1 + rb, 1:65].bitcast(F32R),
                        yv[:, ra:rb, :],
                        ACTF.Silu, bias=b_t[cb][:], scale=s_t[cb][:])
                xp.append(x)

            # ---- fused_red = proj(reshape(fused)) --------------------
            t_sb = big.tile([128, NPIX], F32, tag="mid16")
            for pt in range(8):
                ps = ps8.tile([128, 512], F32, tag="ps")
                for icb in range(2):
                    nc.tensor.matmul(
                        ps[:], wrs_sb[:, icb, :].bitcast(F32R),
                        fused[icb][:, pt * 512:(pt + 1) * 512].bitcast(F32R),
                        start=(icb == 0), stop=(icb == 1))
                nc.scalar.copy(t_sb[:, pt * 512:(pt + 1) * 512].bitcast(F32R), ps[:])
            fr = []
            for cb in range(2):
                f = big.tile([128, NPIX], F32, tag="mid16")
                for pt in range(8):
                    ps = ps8.tile([128, 512], F32, tag="ps")
                    nc.tensor.matmul(
                        ps[:], wpr_sb[:, cb * 128:(cb + 1) * 128].bitcast(F32R),
                        t_sb[:, pt * 512:(pt + 1) * 512].bitcast(F32R))
                    nc.scalar.copy(f[:, pt * 512:(pt + 1) * 512], ps[:])
                fr.append(f)

            # ---- dynamic filter + final add --------------------------
            # fp32r matmuls need even free sizes: every region computed as a
            # uniform 22x22 window (overlapping a row/col into the neighbor
            # band with this region's weights); the final add consumes only
            # the correct sub-rectangle. Output DMA'd per row-band so the
            # store overlaps later bands' compute.
            DBANDS = [(0, 22, 0, 0), (22, 21, 22, 0), (43, 21, 42, 1)]
            for ry, (r0, nr, gr, orow) in enumerate(DBANDS):
                for rx, (c0, ncc, gc, ocol) in enumerate(DBANDS):
                    reg = ry * 3 + rx
                    pds = [ps8.tile([128, 484], F32, tag="ps",
                                    name=f"pd{reg}_{i}") for i in range(2)]
                    for tap in range(9):
                        dy, dx = tap // 3, tap % 3
                        rk = reg * 9 + tap
                        idt = idp.tile([128, 128], F32, tag="idt",
                                       name=f"idt{rk}")
                        if tap % 2 == 0:
                            nc.vector.tensor_scalar_mul(
                                idt[:].bitcast(F32R), eye_sb[:],
                                kbc[:, rk:rk + 1])
                        else:
                            nc.scalar.mul(idt[:].bitcast(F32R), eye_sb[:],
                                          kbc[:, rk:rk + 1])
                        for cb in range(2):
                            nc.tensor.matmul(
                                pds[cb][:], idt[:].bitcast(F32R),
                                xp[cb][:, gr + dy:gr + dy + 22,
                                       gc + dx:gc + dx + 22].bitcast(F32R),
                                start=(tap == 0), stop=(tap == 8))
                    for cb in range(2):
                        fv = fr[cb][:].rearrange("p (h w) -> p h w", h=H)
                        pv = pds[cb][:].rearrange("p (a b) -> p a b", a=22)
                        nc.vector.tensor_add(
                            fv[:, r0:r0 + nr, c0:c0 + ncc],
                            pv[:, orow:orow + nr, ocol:ocol + ncc],
                            fv[:, r0:r0 + nr, c0:c0 + ncc])
                for cb in range(2):
                    dma(outd[cb * 128:(cb + 1) * 128, r0 * 64:(r0 + nr) * 64],
                        fr[cb][:, r0 * 64:(r0 + nr) * 64])

    nc.compile()
    return nc


def _prep_inputs(inputs):
    """Host-side parameter folding + per-core input maps."""
    f = np.float32
    c4r = np.asarray(inputs["c4"], f).reshape(B, C4, H, W)
    c4 = np.zeros((B, C4, 66, 66), f)
    c4[:, :, 1:65, 1:65] = c4r
    c4 = c4.reshape(B, C4, 66 * 66)
    c5 = np.ascontiguousarray(inputs["c5"], f).reshape(B, C5, 1024)
    wc4 = np.ascontiguousarray(
        np.transpose(np.asarray(inputs["w_c4_proc"], f).reshape(OC, C4, 9),
                     (1, 2, 0)))                      # (ic, tap, oc)
    wc1 = np.ascontiguousarray(np.asarray(inputs["w_conv1"], f).reshape(OC, C5).T)
    wtf = np.ascontiguousarray(np.asarray(inputs["w_to_fuse"], f).reshape(OC, C4).T)
    wrs = np.ascontiguousarray(np.asarray(inputs["w_reshape"], f).reshape(FR, C4).T)
    wpr = np.ascontiguousarray(np.asarray(inputs["w_proj"], f).reshape(OC, FR).T)
    w4 = np.asarray(inputs["w_sim4"], f).reshape(64, C4)
    w5 = np.asarray(inputs["w_sim5"], f).reshape(64, C5)
    mt = np.ascontiguousarray(w5.T @ w4)              # (c5, c4) = (W4^T W5)^T
    sig = 1.0 / (1.0 + np.exp(-np.asarray(inputs["mask_raw"], np.float64)))
    fac = np.array([P5FAC[i] * P5FAC[j] for i in range(3) for j in range(3)],
                   np.float64)
    sgp = (sig * fac / (484.0 * 484.0)).astype(f)
    maps = []
    shared = dict(
        wc4t=wc4, wc1t=wc1, wtft=wtf, wrst=wrs, wprt=wpr, mt=mt,
        w1=np.ascontiguousarray(np.asarray(inputs["kg_w1"], f).reshape(HID)),
        b1=np.ascontiguousarray(np.asarray(inputs["kg_b1"], f)),
        w2t=np.ascontiguousarray(np.asarray(inputs["kg_w2"], f).T),
        b2t=np.ascontiguousarray(np.tile(np.asarray(inputs["kg_b2"], f), (K2, 1))),
        sgp=sgp,
        gam=np.ascontiguousarray(np.asarray(inputs["bn_gamma"], f)),
        bet=np.ascontiguousarray(np.asarray(inputs["bn_beta"], f)),
        i128=np.eye(128, dtype=f),
    )
    for b in range(B):
        m = dict(shared)
        m["c4"] = np.ascontiguousarray(c4[b])
        m["c5"] = np.ascontiguousarray(c5[b])
        maps.append(m)
    return maps


def _run(inputs, trace=False):
    if "nc" not in _CACHE:
        _CACHE["nc"] = _build()
    nc = _CACHE["nc"]
    maps = _prep_inputs(inputs)
    return run_bass_kernel_spmd(nc, maps, list(range(NCORES)), trace=trace)


def kernel(**inputs) -> np.ndarray:
    res = _run(inputs, trace=False)
    out = np.stack([res.results[i]["o_out"] for i in range(NCORES)])
    return out.reshape(B, OC, H, W).astype(np.float32)
